# revision 1
# baseline (speedup 1.0000x reference)
"""Trainium2 Bass kernel for a dense GAT layer (B=4, N=2048, FIN=128, K=4 heads, D=32).

Math (per batch b):
    Wh = (H @ W).reshape(N, K, D)
    s[i,k] = <Wh[i,k,:], a_src[k,:]>;  t[j,k] = <Wh[j,k,:], a_dst[k,:]>
    e[i,j,k] = leaky_relu(s[i,k] + t[j,k], 0.2), masked to -inf where A[i,j] == 0
    alpha = softmax_j(e);  out[i] = sum_j alpha[i,j,k] * Wh[j,k,:]

Kernel reformulation (exact in exact arithmetic):
    exp(lrelu(x)) = max(exp(x), exp(0.2 x)); with x = s_i + t_j both branches are
    rank-1, and the i-side factor exp(0.2 s_i) cancels in the softmax. So with
    G_i = exp(0.8 s_i), H_j = exp(0.8 t_j), F2_j = exp(0.2 t_j), m = (A > 0):
        w[j,i]   = m[i,j] * max(G_i * H_j, 1)
        out[i,:] = (sum_j w[j,i] * F2_j * Wh[j,:]) / (sum_j w[j,i] * F2_j)
    Scores live in transposed [j (partitions), i (free)] layout so the
    j-contraction runs on the tensor engine with PSUM accumulation; appending F2
    as an extra column of the stationary operand yields the denominators free.

Sharding: 8 cores = 4 batches x 2 row-halves. The host rotates each core's H
rows / A columns so its own query rows are always local rows 0..1023 (keeps the
SPMD program identical across cores), and ships H and A pre-transposed so the
device needs no fp32 transposes for them.
"""

import numpy as np
from contextlib import ExitStack

import concourse.bacc as bacc
import concourse.mybir as mybir
import concourse.tile as tile
from concourse.bass_utils import run_bass_kernel_spmd

B, N, FIN = 4, 2048, 128
KH, DH = 4, 32
P = 128
NI = 1024  # query rows per core
JT = N // P  # 16 j-chunks
NIB = 2  # i-blocks per core
IBS = NI // NIB  # 512
ICN = IBS // P  # 4 i-chunks of 128 per block
JP = 2  # j-chunks paired per mask-multiply op

f32 = mybir.dt.float32
bf16 = mybir.dt.bfloat16

_CACHE = {}


def _build_program():
    nc = bacc.Bacc("TRN2", target_bir_lowering=False, debug=False)

    def din(name, shape, dtype=f32):
        return nc.dram_tensor(name, list(shape), dtype, kind="ExternalInput").ap()

    wu_d = din("wu", (P, 8 + P))       # tiny tensor: first DMA, PE warm-up fodder
    AT_d = din("AslabT", (N, NI))      # A slab transposed: [j, i]
    sel_d = din("sel", (KH, KH * P))   # head-selector for the Gb0 broadcast
    gscr_d = nc.dram_tensor("gscr", [KH, NI], f32).ap()  # Grow bounce for bcast
    CPW = KH * DH + 2 * KH + P + N  # [W | Ssrc | Sdst | ident | HT]
    cpack_d = din("cpack", (P, CPW))
    oaux_d = nc.dram_tensor(
        "oaux", [NIB, KH, DH + 1, IBS], f32, kind="ExternalOutput"
    ).ap()

    Exp = mybir.ActivationFunctionType.Exp
    Sign = mybir.ActivationFunctionType.Sign
    Copy = mybir.ActivationFunctionType.Copy
    MULT = mybir.AluOpType.mult
    MAX = mybir.AluOpType.max

    with tile.TileContext(nc) as tc, ExitStack() as ctx:
        const = ctx.enter_context(tc.tile_pool(name="const", bufs=1))
        big = ctx.enter_context(tc.tile_pool(name="big", bufs=1))
        dbuf = ctx.enter_context(tc.tile_pool(name="dbuf", bufs=2))
        astg = ctx.enter_context(tc.tile_pool(name="astg", bufs=3))
        work = ctx.enter_context(tc.tile_pool(name="work", bufs=2))
        small = ctx.enter_context(tc.tile_pool(name="small", bufs=2))
        ps = ctx.enter_context(tc.tile_pool(name="ps", bufs=3, space="PSUM"))
        pspv = ctx.enter_context(tc.tile_pool(name="pspv", bufs=1, space="PSUM"))

        # ---- constants / inputs ----
        wu = const.tile([P, 8 + P], f32, tag="wu")
        nc.sync.dma_start(wu[:], wu_d[:])
        cpack = const.tile([P, CPW], f32, tag="cpack")
        nc.sync.dma_start(cpack[:], cpack_d[:])
        sel = const.tile([KH, KH * P], f32, tag="sel")
        nc.sync.dma_start(sel[:], sel_d[:])
        sbW = cpack[:, 0:P]
        ssrc = cpack[:, P:P + KH]
        sdst = cpack[:, P + KH:P + 2 * KH]
        ident = cpack[:, P + 2 * KH:2 * P + 2 * KH]
        HT = cpack[:, 2 * P + 2 * KH:]  # [fin, n]
        # PE warm-up: junk transposes on the tiny first tensor so the HAM
        # clock ramps while the big input DMA is still in flight
        for _ in range(24):
            pwu = ps.tile([P, 512], f32, tag="stg")
            nc.tensor.transpose(pwu[0:8, 0:P], wu[:, 0:8], wu[:, 8:8 + P])

        # ---- mask compare helpers ----
        IS_GT = mybir.AluOpType.is_gt

        # ---- stage 0a: WhT and the chain gating the y-scores (srow/Grow/Gb) ----
        WhT = big.tile([P, N], f32, tag="WhT")  # [kd, n] = (H @ W).T
        for q in range(N // 512):
            pw = ps.tile([P, 512], f32, tag="stg")
            nc.tensor.matmul(pw[:], sbW, HT[:, q * 512:(q + 1) * 512],
                             start=True, stop=True)
            nc.scalar.copy(WhT[:, q * 512:(q + 1) * 512], pw[:])

        srow = big.tile([KH, NI], f32, tag="srow")  # s in [k, i] rows (own rows)
        for q in range(NI // 512):
            ps3 = ps.tile([P, 512], f32, tag="stg")
            nc.tensor.matmul(ps3[0:KH, :], ssrc, WhT[:, q * 512:(q + 1) * 512],
                             start=True, stop=True)
            nc.scalar.copy(srow[:, q * 512:(q + 1) * 512], ps3[0:KH, :])

        Grow = big.tile([KH, NI], f32, tag="Grow")  # exp(0.8 s)
        nc.scalar.activation(Grow[:], srow[:], Exp, scale=0.8)
        nc.sync.dma_start(gscr_d[:], Grow[:])  # bounce to DRAM for broadcast reads

        # G broadcast rows per i-block via stride-0 DRAM reads:
        # Gb[:, k, i] = exp(0.8 s)[k, i] replicated across partitions
        def g_block(ib):
            isl = slice(ib * IBS, (ib + 1) * IBS)
            Gb = dbuf.tile([P, KH, IBS], f32, tag="Gb", name=f"Gb{ib}")
            for k in range(KH):
                nc.sync.dma_start(Gb[:, k, :], gscr_d[k, isl].partition_broadcast(P))
            return Gb

        # block 0 via PE matmul (PE is prologue-idle and this skips the DRAM
        # round-trip latency); block 1 via the DMA broadcast above
        Gb0 = dbuf.tile([P, KH, IBS], f32, tag="Gb", name="Gb0")
        for k in range(KH):
            pg = ps.tile([P, 512], f32, tag="stg")
            nc.tensor.matmul(pg[:], sel[:, k * P:(k + 1) * P], Grow[:, 0:IBS],
                             start=True, stop=True)
            nc.scalar.copy(Gb0[:, k, :], pg[:])

        # ---- stage 0b (per j-chunk): t-factors, PV stationary, and the mask ----
        # Hcol/F2col = exp(0.8 t)/exp(0.2 t) straight from the t PSUM tile;
        # whf[jt][:, k, :] = [Wh_k * F2 | F2]; mask compare on DVE (block 0,
        # prologue-idle) and ACT (block 1).
        Hcol = big.tile([P, JT, KH], f32, tag="Hcol")
        F2col = big.tile([P, JT, KH], f32, tag="F2col")
        whf = []
        mT0, mT1 = [], []
        m0 = m1 = None
        for jt in range(JT):
            pt2 = ps.tile([P, 512], f32, tag="stg")
            nc.tensor.matmul(pt2[:, 0:KH], WhT[:, jt * P:(jt + 1) * P], sdst,
                             start=True, stop=True)
            nc.scalar.activation(Hcol[:, jt, :], pt2[:, 0:KH], Exp, scale=0.8)
            nc.scalar.activation(F2col[:, jt, :], pt2[:, 0:KH], Exp, scale=0.2)

            wt = big.tile([P, KH, DH + 1], f32, tag=f"whf{jt}", name=f"whf{jt}")
            pn = ps.tile([P, 512], f32, tag="stg")
            nc.tensor.transpose(pn[:, 0:P], WhT[:, jt * P:(jt + 1) * P], ident)
            for k in range(KH):
                nc.scalar.activation(
                    wt[:, k, 0:DH], pn[:, k * DH:(k + 1) * DH], Copy,
                    scale=F2col[:, jt, k:k + 1],
                )
            nc.scalar.copy(wt[:, :, DH:DH + 1], F2col[:, jt, :, None])
            whf.append(wt)

            r = jt % JP
            if r == 0:
                m0 = dbuf.tile([P, JP, IBS], bf16, tag="mTp", bufs=2 * (JT // JP),
                               name=f"mT0_{jt // JP}")
                m1 = dbuf.tile([P, JP, IBS], bf16, tag="mTp", bufs=2 * (JT // JP),
                               name=f"mT1_{jt // JP}")
                mT0.append(m0)
                mT1.append(m1)
            aT = astg.tile([P, NI], f32, tag="aT")
            nc.sync.dma_start(aT[:], AT_d[jt * P:(jt + 1) * P, :])
            nc.vector.tensor_scalar(m0[:, r, :], aT[:, 0:IBS], 0.0, None, IS_GT)
            nc.scalar.activation(m1[:, r, :], aT[:, IBS:NI], Sign)

        Gb1 = g_block(1)

        # ---- main loop over i-blocks ----
        for ib, (mTs, Gb) in enumerate([(mT0, Gb0), (mT1, Gb1)]):
            # PV accumulators (transposed): per head [33, i] = [WhF2|F2].T @ w
            # one full PSUM bank per head -> single live accumulation group/bank
            pv = [
                pspv.tile([DH + 1, IBS], f32, tag=f"pv{k}", name=f"pv{k}_{ib}")
                for k in range(KH)
            ]

            for jp in range(JT // JP):
                y8 = work.tile([P, JP, KH, IBS], f32, tag="y8")
                for r in range(JP):
                    jt = jp * JP + r
                    for k in range(KH):
                        eng = nc.vector if k < 2 else nc.gpsimd
                        eng.tensor_scalar(
                            y8[:, r, k, :], Gb[:, k, :], Hcol[:, jt, k:k + 1], 1.0,
                            MULT, MAX,
                        )
                for r in range(JP):
                    nc.vector.tensor_mul(
                        y8[:, r], y8[:, r],
                        mTs[jp][:, r, None, :].broadcast_to((P, KH, IBS)),
                    )
                for r in range(JP):
                    jt = jp * JP + r
                    for k in range(KH):
                        nc.tensor.matmul(
                            pv[k][:],
                            whf[jt][:, k, :],
                            y8[:, r, k, :],
                            start=(jt == 0),
                            stop=(jt == JT - 1),
                        )

            # epilogue: ship the raw [33, i] accumulators (numerators + the
            # denominator row); the host performs the divide and un-transpose
            otT = small.tile([DH + 1, KH, IBS], f32, tag="otT")
            for k in range(KH):
                if k < 2:
                    nc.scalar.copy(otT[:, k, :], pv[k][:])
                else:
                    nc.vector.tensor_copy(otT[:, k, :], pv[k][:])
            nc.sync.dma_start(oaux_d[ib].rearrange("k d i -> d k i"), otT[:])

    nc.compile()
    return nc


def _host_prep(H, A, W, a_src, a_dst):
    """Build the 8 per-core input maps (layout prep only)."""
    Ssrc = np.zeros((FIN, KH), np.float32)
    Sdst = np.zeros((FIN, KH), np.float32)
    for k in range(KH):
        Ssrc[k * DH:(k + 1) * DH, k] = a_src[k]
        Sdst[k * DH:(k + 1) * DH, k] = a_dst[k]

    in_maps = []
    for c in range(8):
        b, half = divmod(c, 2)
        i0 = half * NI
        HbT = np.roll(H[b], -i0, axis=0).T.astype(np.float32)
        AslabT = np.ascontiguousarray(
            np.roll(A[b, i0:i0 + NI, :], -i0, axis=1).T
        ).astype(np.float32)
        cpack = np.concatenate(
            [W.astype(np.float32), Ssrc, Sdst, np.eye(P, dtype=np.float32), HbT],
            axis=1,
        )
        sel = np.zeros((KH, KH * P), np.float32)
        for k in range(KH):
            sel[k, k * P:(k + 1) * P] = 1.0
        wu_host = np.ones((P, 8 + P), np.float32)
        wu_host[:, 8:] = np.eye(P, dtype=np.float32)
        in_maps.append({
            "AslabT": AslabT,
            "cpack": np.ascontiguousarray(cpack),
            "wu": wu_host,
            "sel": sel,
        })
    return in_maps


def kernel(H, A, W, a_src, a_dst, _want_results=False, _trace=False):
    H = np.asarray(H); A = np.asarray(A); W = np.asarray(W)
    a_src = np.asarray(a_src); a_dst = np.asarray(a_dst)

    if "nc" not in _CACHE:
        _CACHE["nc"] = _build_program()
    nc = _CACHE["nc"]

    in_maps = _host_prep(H, A, W, a_src, a_dst)
    res = run_bass_kernel_spmd(nc, in_maps, list(range(8)), trace=_trace)

    out = np.empty((B, N, KH * DH), np.float32)
    for c in range(8):
        b, half = divmod(c, 2)
        i0 = half * NI
        aux = res.results[c]["oaux"]  # [NIB, KH, DH+1, IBS]
        slab = aux[:, :, 0:DH, :] / aux[:, :, DH:DH + 1, :]
        # [ib, k, d, i] -> rows (ib*IBS + i), cols (k*DH + d)
        out[b, i0:i0 + NI, :] = (
            slab.transpose(0, 3, 1, 2).reshape(NI, KH * DH)
        )
    if _want_results:
        return out, res
    return out



# revision 7
# speedup vs baseline: 1.7529x; 1.7529x over previous
"""Trainium2 Bass kernel for a dense GAT layer (B=4, N=2048, FIN=128, K=4 heads, D=32).

Math (per batch b):
    Wh = (H @ W).reshape(N, K, D)
    s[i,k] = <Wh[i,k,:], a_src[k,:]>;  t[j,k] = <Wh[j,k,:], a_dst[k,:]>
    e[i,j,k] = leaky_relu(s[i,k] + t[j,k], 0.2), masked to -inf where A[i,j] == 0
    alpha = softmax_j(e);  out[i] = sum_j alpha[i,j,k] * Wh[j,k,:]

Reformulation (exact): with x = s_i + t_j,
    exp(lrelu(x)) = max(exp x, exp 0.2x); the i-side factor exp(0.2 s_i)
    cancels in the softmax.  With G = exp(0.8 s_i), HF_j = exp(t_j),
    F2_j = exp(0.2 t_j), m = (A > 0):
        v[j,i,k] = max(G_ik * HF_jk, F2_jk)          (= F2 * max(GH, 1))
        y[j,i,k] = v * m[j,i]
        num[i,k,:] = sum_j y * Wh[j,k,:];  den[i,k] = sum_j y
        out = num / den
    F2 is folded into the tensor-scalar pass (HF = exp t as the multiplier,
    F2 as the max floor), so the PV stationary is raw [Wh | 1] and the
    denominator falls out of the ones column.

All matmul operands and the score volume are bf16 (PE 1 cyc/row instead of
fp32's 4; DVE 4x mode on the tensor-scalar pass, 2x on the mask multiply).
The mask ships from the host as bf16 {0,1}, halving HBM traffic vs fp32 A.

Sharding: 8 cores = 4 batches x 2 row-halves (i-slabs of 1024); no
collectives.  The host rotates H rows / A columns so each core's query rows
are local 0..1023 (identical SPMD program), and ships H pre-transposed.
"""

import numpy as np
import ml_dtypes
from contextlib import ExitStack

import concourse.bacc as bacc
import concourse.mybir as mybir
import concourse.tile as tile
from concourse.bass_utils import run_bass_kernel_spmd

B, N, FIN = 4, 2048, 128
KH, DH = 4, 32
P = 128
NI = 1024           # query rows per core
JT = N // P         # 16 j-chunks
WUC = 8 + 512       # warmup tile cols

f32 = mybir.dt.float32
bf16 = mybir.dt.bfloat16
BF = ml_dtypes.bfloat16

_CACHE = {}

# pass1 engine assignment per (jt, k): 'v' = DVE tensor_scalar,
# 'p' = Pool tensor_scalar, 'a' = ACT relu+exp chain
PASS1_ENG = {}
for _jt in range(JT):
    PASS1_ENG[(_jt, 0)] = "v"
    PASS1_ENG[(_jt, 1)] = "v" if _jt < 9 else "p"
    PASS1_ENG[(_jt, 2)] = "p"
    PASS1_ENG[(_jt, 3)] = "a" if _jt < 15 else "v"

JW0 = 20   # initial warmup junk matmuls
JWL = 4    # keep-warm junk matmuls per jt in the main loop


def _build_program():
    nc = bacc.Bacc("TRN2", target_bir_lowering=False, debug=False)

    def din(name, shape, dtype):
        return nc.dram_tensor(name, list(shape), dtype, kind="ExternalInput").ap()

    wu_d = din("wu", (P, WUC), bf16)      # tiny first DMA: PE warmup fodder
    CPW = P + 2 * KH + P + N              # [W | Ssrc | Sdst | identb | HT]
    cpack_d = din("cpack", (P, CPW), bf16)
    mT_d = din("maskT", (N, NI), bf16)    # mask (A>0) transposed: [j, i]
    gscr_d = nc.dram_tensor("gscr", [36, NI], bf16).ap()  # [Grow; pad; Sraw] bounce
    oaux_d = nc.dram_tensor("oaux", [4, 97, 512], f32, kind="ExternalOutput").ap()

    Exp = mybir.ActivationFunctionType.Exp
    Relu = mybir.ActivationFunctionType.Relu
    Copy = mybir.ActivationFunctionType.Copy
    MULT = mybir.AluOpType.mult
    MAX = mybir.AluOpType.max

    with tile.TileContext(nc) as tc, ExitStack() as ctx:
        const = ctx.enter_context(tc.tile_pool(name="const", bufs=1))
        big = ctx.enter_context(tc.tile_pool(name="big", bufs=1))
        vwork = ctx.enter_context(tc.tile_pool(name="vwork", bufs=3))
        ywork = ctx.enter_context(tc.tile_pool(name="ywork", bufs=3))
        small = ctx.enter_context(tc.tile_pool(name="small", bufs=2))
        ps = ctx.enter_context(tc.tile_pool(name="ps", bufs=2, space="PSUM"))
        pst = ctx.enter_context(tc.tile_pool(name="pst", bufs=1, space="PSUM"))
        pspv = ctx.enter_context(tc.tile_pool(name="pspv", bufs=1, space="PSUM"))

        # ---- constants / inputs ----
        wu = const.tile([P, WUC], bf16, tag="wu")
        nc.sync.dma_start(wu[:], wu_d[:])
        cpack = const.tile([P, CPW], bf16, tag="cpack")
        nc.sync.dma_start(cpack[:], cpack_d[:])
        sbW = cpack[:, 0:P]
        ssrc = cpack[:, P:P + KH]
        sdst = cpack[:, P + KH:P + 2 * KH]
        identb = cpack[:, P + 2 * KH:2 * P + 2 * KH]
        HT = cpack[:, 2 * P + 2 * KH:]  # [fin, n]

        # mask tiles: one big persistent buffer, 16 chunk DMAs
        maskT = big.tile([P, JT, NI], bf16, tag="maskT")
        for jt in range(JT):
            nc.sync.dma_start(maskT[:, jt, :], mT_d[jt * P:(jt + 1) * P, :])

        # PE warm-up junk: ramps the PE clock while input DMAs fly
        for _ in range(JW0):
            pj = ps.tile([P, 512], f32, tag="stg")
            nc.tensor.matmul(pj[0:8, :], wu[:, 0:8], wu[:, 8:8 + 512],
                             start=True, stop=True)

        # ---- WhT = (H @ W).T in bf16; first 2 chunks feed srow ASAP ----
        WhT = big.tile([P, N], bf16, tag="WhT")
        for q in range(N // 512):
            pw = ps.tile([P, 512], f32, tag="stg")
            nc.tensor.matmul(pw[:], sbW, HT[:, q * 512:(q + 1) * 512],
                             start=True, stop=True)
            eng = nc.vector if q % 2 == 0 else nc.scalar
            if q % 2 == 0:
                nc.vector.tensor_copy(WhT[:, q * 512:(q + 1) * 512], pw[:])
            else:
                nc.scalar.copy(WhT[:, q * 512:(q + 1) * 512], pw[:])

            if q == 1:
                # srow for own rows (cols 0..1023) -> Grow/Sraw -> DRAM bounce
                GrowSb = small.tile([36, NI], bf16, tag="GrowSb", bufs=1)
                for h in range(2):
                    psr = ps.tile([P, 512], f32, tag="stg")
                    nc.tensor.matmul(psr[0:KH, :], ssrc,
                                     WhT[:, h * 512:(h + 1) * 512],
                                     start=True, stop=True)
                    nc.scalar.activation(GrowSb[0:KH, h * 512:(h + 1) * 512],
                                         psr[0:KH, :], Exp, scale=0.8)
                    nc.scalar.copy(GrowSb[32:32 + KH, h * 512:(h + 1) * 512],
                                   psr[0:KH, :])
                nc.sync.dma_start(gscr_d[:], GrowSb[:])

        # G / S broadcast tiles via stride-0 DRAM reads
        Gball = big.tile([P, KH, NI], bf16, tag="Gball")
        Sball = big.tile([P, KH, NI], bf16, tag="Sball")
        for k in range(KH):
            nc.sync.dma_start(Gball[:, k, :], gscr_d[k, :].partition_broadcast(P))
            nc.sync.dma_start(Sball[:, k, :],
                              gscr_d[32 + k, :].partition_broadcast(P))

        # ---- t-scores for all j: pst [128, JT*KH]; HF = exp(t), F2 = exp(0.2t)
        ptt = pst.tile([P, JT * KH], f32, tag="ptt")
        for jt in range(JT):
            nc.tensor.matmul(ptt[:, jt * KH:(jt + 1) * KH],
                             WhT[:, jt * P:(jt + 1) * P], sdst,
                             start=True, stop=True, skip_group_check=True)
        HFcol = big.tile([P, JT * KH], f32, tag="HFcol")
        F2col = big.tile([P, JT * KH], f32, tag="F2col")
        tcol = big.tile([P, JT * KH], f32, tag="tcol")
        t02col = big.tile([P, JT * KH], f32, tag="t02col")
        nc.scalar.activation(HFcol[:], ptt[:], Exp, scale=1.0)
        nc.scalar.activation(F2col[:], ptt[:], Exp, scale=0.2)
        nc.scalar.copy(tcol[:], ptt[:])
        nc.scalar.activation(t02col[:], ptt[:], Copy, scale=0.2)

        # ---- whf[jt] = [Wh | 1] per head, bf16, via PE transpose ----
        whf = []
        for jt in range(JT):
            pn = ps.tile([P, 512], bf16, tag="stgb", bufs=1)
            nc.tensor.transpose(pn[:, 0:P], WhT[:, jt * P:(jt + 1) * P], identb)
            wt = big.tile([P, KH, DH + 1], bf16, tag=f"whf{jt}", name=f"whf{jt}")
            nc.gpsimd.memset(wt[:, :, DH:DH + 1], 1.0)
            nc.scalar.copy(
                wt[:, :, 0:DH],
                pn[:, 0:P].rearrange("p (k d) -> p k d", k=KH),
            )
            whf.append(wt)

        # ---- main loop over j-chunks ----
        pv = [pspv.tile([97, 512], f32, tag=f"pv{q}", name=f"pv{q}")
              for q in range(4)]

        for jt in range(JT):
            v = vwork.tile([P, KH, NI], bf16, tag="v")
            for k in range(KH):
                e = PASS1_ENG[(jt, k)]
                c = jt * KH + k
                if e == "a":
                    rt = small.tile([P, NI], bf16, tag="rt")
                    nc.scalar.activation(rt[:], Sball[:, k, :], Relu,
                                         bias=tcol[:, c:c + 1], scale=1.0)
                    nc.scalar.activation(v[:, k, :], rt[:], Exp,
                                         bias=t02col[:, c:c + 1], scale=0.8)
                else:
                    eng = nc.vector if e == "v" else nc.gpsimd
                    eng.tensor_scalar(v[:, k, :], Gball[:, k, :],
                                      HFcol[:, c:c + 1], F2col[:, c:c + 1],
                                      MULT, MAX)
            y8 = ywork.tile([P, KH, NI], bf16, tag="y8")
            nc.vector.tensor_mul(
                y8[:], v[:],
                maskT[:, jt, None, :].broadcast_to((P, KH, NI)),
            )
            for ib in range(2):
                isl = slice(ib * 512, (ib + 1) * 512)
                for k in range(KH):
                    q, r = divmod(k, 2)
                    nc.tensor.matmul(
                        pv[q * 2 + ib][r * 64:r * 64 + 33, :],
                        whf[jt][:, k, :],
                        y8[:, k, isl],
                        start=(jt == 0), stop=(jt == JT - 1),
                        skip_group_check=True,
                    )
            for _ in range(JWL):
                pj = ps.tile([P, 512], f32, tag="stg")
                nc.tensor.matmul(pj[0:8, :], wu[:, 0:8], wu[:, 8:8 + 512],
                                 start=True, stop=True)

        # ---- epilogue: raw accumulators out; host does divide/transpose ----
        for q in range(4):
            ot = small.tile([97, 512], f32, tag="ot")
            if q % 2 == 0:
                nc.vector.tensor_copy(ot[:], pv[q][:])
            else:
                nc.scalar.copy(ot[:], pv[q][:])
            nc.sync.dma_start(oaux_d[q], ot[:])

    nc.compile()
    return nc


def _host_prep(H, A, W, a_src, a_dst):
    """Build the 8 per-core input maps (layout prep + dtype casts only)."""
    Ssrc = np.zeros((FIN, KH), np.float32)
    Sdst = np.zeros((FIN, KH), np.float32)
    for k in range(KH):
        Ssrc[k * DH:(k + 1) * DH, k] = a_src[k]
        Sdst[k * DH:(k + 1) * DH, k] = a_dst[k]

    wu_host = np.ones((P, WUC), np.float32)
    wu_host[:, 8:8 + P] = np.eye(P, dtype=np.float32)
    wu_host = wu_host.astype(BF)

    in_maps = []
    for c in range(8):
        b, half = divmod(c, 2)
        i0 = half * NI
        HbT = np.roll(H[b], -i0, axis=0).T  # [FIN, N], j rolled
        maskT = np.ascontiguousarray(
            (np.roll(A[b, i0:i0 + NI, :], -i0, axis=1) > 0).T
        ).astype(BF)
        cpack = np.concatenate(
            [W.astype(np.float32), Ssrc, Sdst, np.eye(P, dtype=np.float32), HbT],
            axis=1,
        ).astype(BF)
        in_maps.append({
            "wu": wu_host,
            "cpack": np.ascontiguousarray(cpack),
            "maskT": maskT,
        })
    return in_maps


def kernel(H, A, W, a_src, a_dst, _want_results=False, _trace=False):
    H = np.asarray(H); A = np.asarray(A); W = np.asarray(W)
    a_src = np.asarray(a_src); a_dst = np.asarray(a_dst)

    if "nc" not in _CACHE:
        _CACHE["nc"] = _build_program()
    nc = _CACHE["nc"]

    in_maps = _host_prep(H, A, W, a_src, a_dst)
    res = run_bass_kernel_spmd(nc, in_maps, list(range(8)), trace=_trace)

    out = np.empty((B, N, KH * DH), np.float32)
    for c in range(8):
        b, half = divmod(c, 2)
        i0 = half * NI
        aux = res.results[c]["oaux"]  # [4, 97, 512] f32
        for q in range(4):
            p, ibb = divmod(q, 2)
            r0 = i0 + ibb * 512
            for h2 in range(2):
                k = 2 * p + h2
                blk = aux[q, h2 * 64:h2 * 64 + 33]  # [33, 512]
                out[b, r0:r0 + 512, k * DH:(k + 1) * DH] = (
                    blk[0:DH] / blk[DH:DH + 1]
                ).T
        if _want_results:
            pass
    if _want_results:
        return out, res
    return out


# revision 8
# speedup vs baseline: 1.8090x; 1.0320x over previous
"""Trainium2 Bass kernel for a dense GAT layer (B=4, N=2048, FIN=128, K=4 heads, D=32).

Math (per batch b):
    Wh = (H @ W).reshape(N, K, D)
    s[i,k] = <Wh[i,k,:], a_src[k,:]>;  t[j,k] = <Wh[j,k,:], a_dst[k,:]>
    e[i,j,k] = leaky_relu(s[i,k] + t[j,k], 0.2), masked to -inf where A[i,j] == 0
    alpha = softmax_j(e);  out[i] = sum_j alpha[i,j,k] * Wh[j,k,:]

Reformulation (exact): with x = s_i + t_j,
    exp(lrelu(x)) = max(exp x, exp 0.2x); the i-side factor exp(0.2 s_i)
    cancels in the softmax.  With G = exp(0.8 s_i), HF_j = exp(t_j),
    F2_j = exp(0.2 t_j), m = (A > 0):
        v[j,i,k] = max(G_ik * HF_jk, F2_jk)          (= F2 * max(GH, 1))
        y[j,i,k] = v * m[j,i]
        num[i,k,:] = sum_j y * Wh[j,k,:];  den[i,k] = sum_j y
        out = num / den
    F2 folds into the tensor-scalar pass (HF as multiplier, F2 as max
    floor), so the PV stationary is raw [Wh | 1] and the denominator falls
    out of the ones column.

All matmul operands and the score volume are bf16 (PE 1 cyc/row instead of
fp32's 4; DVE 4x mode on the tensor-scalar pass, 2x on the mask multiply).
The mask ships from the host as bf16 {0,1}.  Engine split per (jt, head) is
table-driven: DVE does the mask TensorTensor (the dominant pass) plus cheap
4x TensorScalars; Pool and ACT (relu+exp chain) absorb the rest of pass 1.

Sharding: 8 cores = 4 batches x 2 row-halves (i-slabs of 1024); no
collectives.  Host rotates H rows / A columns so each core's query rows are
local 0..1023 (identical SPMD program) and ships H pre-transposed.
"""

import numpy as np
import ml_dtypes
from contextlib import ExitStack

import concourse.bacc as bacc
import concourse.mybir as mybir
import concourse.tile as tile
from concourse.bass_utils import run_bass_kernel_spmd

B, N, FIN = 4, 2048, 128
KH, DH = 4, 32
P = 128
NI = 1024           # query rows per core
JT = N // P         # 16 j-chunks
WUC = 8 + 512       # warmup tile cols

f32 = mybir.dt.float32
bf16 = mybir.dt.bfloat16
BF = ml_dtypes.bfloat16

_CACHE = {}

# pass1 engine per (jt, k): 'v' = DVE TS, 'p' = Pool TS, 'a' = ACT relu+exp
PASS1_ENG = {}
for _jt in range(JT):
    PASS1_ENG[(_jt, 0)] = "v"
    PASS1_ENG[(_jt, 1)] = "v"
    PASS1_ENG[(_jt, 2)] = "p"
    PASS1_ENG[(_jt, 3)] = "a"
# jts whose k3 slice of the mask TT runs on Pool instead of DVE
TT_POOL_JT = {3, 6, 9, 12, 14, 15}

JW0 = 8   # initial warmup junk matmuls


def _build_program():
    nc = bacc.Bacc("TRN2", target_bir_lowering=False, debug=False)

    def din(name, shape, dtype):
        return nc.dram_tensor(name, list(shape), dtype, kind="ExternalInput").ap()

    wu_d = din("wu", (P, WUC), bf16)      # tiny first DMA: PE warmup fodder
    CPW = P + 2 * KH + P + N              # [W | Ssrc | Sdst | identb | HT]
    cpack_d = din("cpack", (P, CPW), bf16)
    mT_d = din("maskT", (N, NI), bf16)    # mask (A>0) transposed: [j, i]
    gscr_d = nc.dram_tensor("gscr", [36, NI], bf16).ap()  # [Grow; pad; Sraw]
    oaux_d = nc.dram_tensor("oaux", [4, 97, 512], f32, kind="ExternalOutput").ap()

    Exp = mybir.ActivationFunctionType.Exp
    Relu = mybir.ActivationFunctionType.Relu
    Copy = mybir.ActivationFunctionType.Copy
    MULT = mybir.AluOpType.mult
    MAX = mybir.AluOpType.max

    with tile.TileContext(nc) as tc, ExitStack() as ctx:
        const = ctx.enter_context(tc.tile_pool(name="const", bufs=1))
        big = ctx.enter_context(tc.tile_pool(name="big", bufs=1))
        vwork = ctx.enter_context(tc.tile_pool(name="vwork", bufs=5))
        ywork = ctx.enter_context(tc.tile_pool(name="ywork", bufs=4))
        small = ctx.enter_context(tc.tile_pool(name="small", bufs=2))
        ps = ctx.enter_context(tc.tile_pool(name="ps", bufs=2, space="PSUM"))
        pst = ctx.enter_context(tc.tile_pool(name="pst", bufs=1, space="PSUM"))
        pspv = ctx.enter_context(tc.tile_pool(name="pspv", bufs=1, space="PSUM"))

        # ---- constants / inputs ----
        wu = const.tile([P, WUC], bf16, tag="wu")
        nc.sync.dma_start(wu[:], wu_d[:])
        cpack = const.tile([P, CPW], bf16, tag="cpack")
        nc.sync.dma_start(cpack[:], cpack_d[:])
        sbW = cpack[:, 0:P]
        ssrc = cpack[:, P:P + KH]
        sdst = cpack[:, P + KH:P + 2 * KH]
        identb = cpack[:, P + 2 * KH:2 * P + 2 * KH]
        HT = cpack[:, 2 * P + 2 * KH:]  # [fin, n]

        # mask tiles: one big persistent buffer, 16 chunk DMAs
        maskT = big.tile([P, JT, NI], bf16, tag="maskT")
        for jt in range(JT):
            nc.sync.dma_start(maskT[:, jt, :], mT_d[jt * P:(jt + 1) * P, :])

        # PE warm-up junk while cpack lands
        for _ in range(JW0):
            pj = ps.tile([P, 512], f32, tag="stg")
            nc.tensor.matmul(pj[0:8, :], wu[:, 0:8], wu[:, 8:8 + 512],
                             start=True, stop=True)

        # ---- WhT = (H @ W).T bf16; chunks 0/1 first to unblock srow ----
        WhT = big.tile([P, N], bf16, tag="WhT")
        for q in range(N // 512):
            pw = ps.tile([P, 512], f32, tag="stg")
            nc.tensor.matmul(pw[:], sbW, HT[:, q * 512:(q + 1) * 512],
                             start=True, stop=True)
            nc.vector.tensor_copy(WhT[:, q * 512:(q + 1) * 512], pw[:])

            if q == 1:
                # srow (own rows = cols 0..1023) -> Grow/Sraw -> DRAM bounce
                GrowSb = small.tile([36, NI], bf16, tag="GrowSb", bufs=1)
                for h in range(2):
                    psr = ps.tile([P, 512], f32, tag="stg")
                    nc.tensor.matmul(psr[0:KH, :], ssrc,
                                     WhT[:, h * 512:(h + 1) * 512],
                                     start=True, stop=True)
                    nc.scalar.activation(GrowSb[0:KH, h * 512:(h + 1) * 512],
                                         psr[0:KH, :], Exp, scale=0.8)
                    nc.scalar.copy(GrowSb[32:32 + KH, h * 512:(h + 1) * 512],
                                   psr[0:KH, :])
                nc.sync.dma_start(gscr_d[:], GrowSb[:])
                # broadcast reads issued right away (queue after the write)
                Gball = big.tile([P, KH, NI], bf16, tag="Gball")
                Sball = big.tile([P, KH, NI], bf16, tag="Sball")
                for k in range(KH):
                    nc.sync.dma_start(Gball[:, k, :],
                                      gscr_d[k, :].partition_broadcast(P))
                    nc.sync.dma_start(Sball[:, k, :],
                                      gscr_d[32 + k, :].partition_broadcast(P))

        # ---- t-scores for all j; HF = exp t, F2 = exp 0.2t (+ raw t biases)
        ptt = pst.tile([P, JT * KH], f32, tag="ptt")
        for jt in range(JT):
            nc.tensor.matmul(ptt[:, jt * KH:(jt + 1) * KH],
                             WhT[:, jt * P:(jt + 1) * P], sdst,
                             start=True, stop=True, skip_group_check=True)
        HFcol = big.tile([P, JT * KH], f32, tag="HFcol")
        F2col = big.tile([P, JT * KH], f32, tag="F2col")
        tcol = big.tile([P, JT * KH], f32, tag="tcol")
        t02col = big.tile([P, JT * KH], f32, tag="t02col")
        nc.scalar.activation(HFcol[:], ptt[:], Exp, scale=1.0)
        nc.scalar.activation(F2col[:], ptt[:], Exp, scale=0.2)
        nc.scalar.copy(tcol[:], ptt[:])
        nc.scalar.activation(t02col[:], ptt[:], Copy, scale=0.2)

        # ---- whf[jt] = [Wh | 1] per head, bf16, via PE transpose ----
        # 4 transpose slots in one PSUM tile so the ACT copies pipeline
        pn4 = pst.tile([P, 4, P], bf16, tag="pn4")
        whf = []
        for jt in range(JT):
            nc.tensor.transpose(pn4[:, jt % 4, :], WhT[:, jt * P:(jt + 1) * P],
                                identb)
            wt = big.tile([P, KH, DH + 1], bf16, tag=f"whf{jt}", name=f"whf{jt}")
            nc.gpsimd.memset(wt[:, :, DH:DH + 1], 1.0)
            nc.scalar.copy(
                wt[:, :, 0:DH],
                pn4[:, jt % 4, :].rearrange("p (k d) -> p k d", k=KH),
            )
            whf.append(wt)

        # ---- main loop over j-chunks ----
        pv = [pspv.tile([97, 512], f32, tag=f"pv{q}", name=f"pv{q}")
              for q in range(4)]

        for jt in range(JT):
            v = vwork.tile([P, KH, NI], bf16, tag="v")
            for k in range(KH):
                e = PASS1_ENG[(jt, k)]
                c = jt * KH + k
                if e == "a":
                    rt = small.tile([P, NI], bf16, tag="rt", bufs=3)
                    nc.scalar.activation(rt[:], Sball[:, k, :], Relu,
                                         bias=tcol[:, c:c + 1], scale=1.0)
                    nc.scalar.activation(v[:, k, :], rt[:], Exp,
                                         bias=t02col[:, c:c + 1], scale=0.8)
                else:
                    eng = nc.vector if e == "v" else nc.gpsimd
                    eng.tensor_scalar(v[:, k, :], Gball[:, k, :],
                                      HFcol[:, c:c + 1], F2col[:, c:c + 1],
                                      MULT, MAX)
            y8 = ywork.tile([P, KH, NI], bf16, tag="y8")
            if jt in TT_POOL_JT:
                nc.vector.tensor_mul(
                    y8[:, 0:3], v[:, 0:3],
                    maskT[:, jt, None, :].broadcast_to((P, 3, NI)),
                )
                nc.gpsimd.tensor_mul(y8[:, 3], v[:, 3], maskT[:, jt, :])
            else:
                nc.vector.tensor_mul(
                    y8[:], v[:],
                    maskT[:, jt, None, :].broadcast_to((P, KH, NI)),
                )
            for ib in range(2):
                isl = slice(ib * 512, (ib + 1) * 512)
                for k in range(KH):
                    q, r = divmod(k, 2)
                    nc.tensor.matmul(
                        pv[q * 2 + ib][r * 64:r * 64 + 33, :],
                        whf[jt][:, k, :],
                        y8[:, k, isl],
                        start=(jt == 0), stop=(jt == JT - 1),
                        skip_group_check=True,
                    )

        # ---- epilogue: raw accumulators out; host divides / transposes ----
        for q in range(4):
            ot = small.tile([97, 512], f32, tag="ot")
            if q % 2 == 0:
                nc.vector.tensor_copy(ot[:], pv[q][:])
            else:
                nc.scalar.copy(ot[:], pv[q][:])
            nc.sync.dma_start(oaux_d[q], ot[:])

    nc.compile()
    return nc


def _host_prep(H, A, W, a_src, a_dst):
    """Build the 8 per-core input maps (layout prep + dtype casts only)."""
    Ssrc = np.zeros((FIN, KH), np.float32)
    Sdst = np.zeros((FIN, KH), np.float32)
    for k in range(KH):
        Ssrc[k * DH:(k + 1) * DH, k] = a_src[k]
        Sdst[k * DH:(k + 1) * DH, k] = a_dst[k]

    wu_host = np.ones((P, WUC), np.float32)
    wu_host[:, 8:8 + P] = np.eye(P, dtype=np.float32)
    wu_host = wu_host.astype(BF)

    in_maps = []
    for c in range(8):
        b, half = divmod(c, 2)
        i0 = half * NI
        HbT = np.roll(H[b], -i0, axis=0).T  # [FIN, N], j rolled
        maskT = np.ascontiguousarray(
            (np.roll(A[b, i0:i0 + NI, :], -i0, axis=1) > 0).T
        ).astype(BF)
        cpack = np.concatenate(
            [W.astype(np.float32), Ssrc, Sdst, np.eye(P, dtype=np.float32), HbT],
            axis=1,
        ).astype(BF)
        in_maps.append({
            "wu": wu_host,
            "cpack": np.ascontiguousarray(cpack),
            "maskT": maskT,
        })
    return in_maps


def kernel(H, A, W, a_src, a_dst, _want_results=False, _trace=False):
    H = np.asarray(H); A = np.asarray(A); W = np.asarray(W)
    a_src = np.asarray(a_src); a_dst = np.asarray(a_dst)

    if "nc" not in _CACHE:
        _CACHE["nc"] = _build_program()
    nc = _CACHE["nc"]

    in_maps = _host_prep(H, A, W, a_src, a_dst)
    res = run_bass_kernel_spmd(nc, in_maps, list(range(8)), trace=_trace)

    out = np.empty((B, N, KH * DH), np.float32)
    for c in range(8):
        b, half = divmod(c, 2)
        i0 = half * NI
        aux = res.results[c]["oaux"]  # [4, 97, 512] f32
        for q in range(4):
            p, ibb = divmod(q, 2)
            r0 = i0 + ibb * 512
            for h2 in range(2):
                k = 2 * p + h2
                blk = aux[q, h2 * 64:h2 * 64 + 33]  # [33, 512]
                out[b, r0:r0 + 512, k * DH:(k + 1) * DH] = (
                    blk[0:DH] / blk[DH:DH + 1]
                ).T
    if _want_results:
        return out, res
    return out


# revision 9
# speedup vs baseline: 1.8455x; 1.0202x over previous
"""Trainium2 Bass kernel for a dense GAT layer (B=4, N=2048, FIN=128, K=4 heads, D=32).

Math (per batch b):
    Wh = (H @ W).reshape(N, K, D)
    s[i,k] = <Wh[i,k,:], a_src[k,:]>;  t[j,k] = <Wh[j,k,:], a_dst[k,:]>
    e[i,j,k] = leaky_relu(s[i,k] + t[j,k], 0.2), masked to -inf where A[i,j] == 0
    alpha = softmax_j(e);  out[i] = sum_j alpha[i,j,k] * Wh[j,k,:]

Reformulation (exact): with x = s_i + t_j,
    exp(lrelu(x)) = max(exp x, exp 0.2x); the i-side factor exp(0.2 s_i)
    cancels in the softmax.  With G = exp(0.8 s_i), HF_j = exp(t_j),
    F2_j = exp(0.2 t_j), m = (A > 0):
        v[j,i,k] = max(G_ik * HF_jk, F2_jk)          (= F2 * max(GH, 1))
        y[j,i,k] = v * m[j,i]
        num[i,k,:] = sum_j y * Wh[j,k,:];  den[i,k] = sum_j y
        out = num / den
    F2 folds into the tensor-scalar pass (HF as multiplier, F2 as max
    floor), so the PV stationary is raw [Wh | 1] and the denominator falls
    out of the ones column.  s and t come straight from H via host-side
    W@a_src / W@a_dst folds, so the score chain never waits on Wh.

All matmul operands and the score volume are bf16 (PE 1 cyc/row instead of
fp32's 4; DVE 4x mode on the tensor-scalar pass, 2x on the mask multiply).
The mask ships from the host as bf16 {0,1}.  Engine split per (jt, head) is
table-driven: DVE owns the mask TensorTensor (dominant pass) plus cheap 4x
TensorScalars, Pool takes one head's TensorScalar and some mask slices,
ACT absorbs one head via a relu+exp chain on broadcast s.

Sharding: 8 cores = 4 batches x 2 row-halves (i-slabs of 1024); no
collectives.  Host rotates H rows / A columns so each core's query rows are
local 0..1023 (identical SPMD program) and ships H pre-transposed.
"""

import numpy as np
import ml_dtypes
from contextlib import ExitStack

import concourse.bacc as bacc
import concourse.mybir as mybir
import concourse.tile as tile
from concourse.bass_utils import run_bass_kernel_spmd

B, N, FIN = 4, 2048, 128
KH, DH = 4, 32
P = 128
NI = 1024           # query rows per core
JT = N // P         # 16 j-chunks
WUC = 8 + 512       # warmup tile cols

f32 = mybir.dt.float32
bf16 = mybir.dt.bfloat16
BF = ml_dtypes.bfloat16

_CACHE = {}

# pass1 engine per (jt, k): 'v' = DVE TS, 'p' = Pool TS, 'a' = ACT relu+exp
PASS1_ENG = {}
for _jt in range(JT):
    PASS1_ENG[(_jt, 0)] = "v"
    PASS1_ENG[(_jt, 1)] = "v"
    PASS1_ENG[(_jt, 2)] = "p"
    PASS1_ENG[(_jt, 3)] = "a"
# jts whose k3 slice of the mask TT runs on Pool instead of DVE
TT_POOL_JT = {2, 5, 8, 11, 14}

JW0 = 5   # initial warmup junk matmuls


def _build_program():
    nc = bacc.Bacc("TRN2", target_bir_lowering=False, debug=False)

    def din(name, shape, dtype):
        return nc.dram_tensor(name, list(shape), dtype, kind="ExternalInput").ap()

    wu_d = din("wu", (P, WUC), bf16)      # tiny first DMA: PE warmup fodder
    CPW = P + 2 * KH + P + N              # [W | WSsrc | WSdst | identb | HT]
    cpack_d = din("cpack", (P, CPW), bf16)
    mT_d = din("maskT", (N, NI), bf16)    # mask (A>0) transposed: [j, i]
    gscr_d = nc.dram_tensor("gscr", [36, NI], bf16).ap()  # [Grow; pad; Sraw]
    oaux_d = nc.dram_tensor("oaux", [4, 2, 33, 512], f32,
                            kind="ExternalOutput").ap()

    Exp = mybir.ActivationFunctionType.Exp
    Relu = mybir.ActivationFunctionType.Relu
    Copy = mybir.ActivationFunctionType.Copy
    MULT = mybir.AluOpType.mult
    MAX = mybir.AluOpType.max

    with tile.TileContext(nc) as tc, ExitStack() as ctx:
        const = ctx.enter_context(tc.tile_pool(name="const", bufs=1))
        big = ctx.enter_context(tc.tile_pool(name="big", bufs=1))
        vwork = ctx.enter_context(tc.tile_pool(name="vwork", bufs=5))
        ywork = ctx.enter_context(tc.tile_pool(name="ywork", bufs=4))
        small = ctx.enter_context(tc.tile_pool(name="small", bufs=2))
        ps = ctx.enter_context(tc.tile_pool(name="ps", bufs=2, space="PSUM"))
        pst = ctx.enter_context(tc.tile_pool(name="pst", bufs=1, space="PSUM"))
        pspv = ctx.enter_context(tc.tile_pool(name="pspv", bufs=1, space="PSUM"))

        # ---- inputs; DMA queue order is the critical schedule ----
        wu = const.tile([P, WUC], bf16, tag="wu")
        nc.sync.dma_start(wu[:], wu_d[:])
        cpack = const.tile([P, CPW], bf16, tag="cpack")
        nc.sync.dma_start(cpack[:], cpack_d[:])
        sbW = cpack[:, 0:P]
        wssrc = cpack[:, P:P + KH]
        wsdst = cpack[:, P + KH:P + 2 * KH]
        identb = cpack[:, P + 2 * KH:2 * P + 2 * KH]
        HT = cpack[:, 2 * P + 2 * KH:]  # [fin, n]

        maskT = big.tile([P, JT, NI], bf16, tag="maskT")
        for jt in range(2):  # first two mask chunks up front
            nc.sync.dma_start(maskT[:, jt, :], mT_d[jt * P:(jt + 1) * P, :])

        # PE warm-up junk while cpack lands
        for _ in range(JW0):
            pj = ps.tile([P, 512], f32, tag="stg")
            nc.tensor.matmul(pj[0:8, :], wu[:, 0:8], wu[:, 8:8 + 512],
                             start=True, stop=True)

        # ---- srow/t straight from HT (host folded W into a_src/a_dst) ----
        GrowSb = small.tile([36, NI], bf16, tag="GrowSb", bufs=1)
        for h in range(2):
            psr = ps.tile([P, 512], f32, tag="stg")
            nc.tensor.matmul(psr[0:KH, :], wssrc,
                             HT[:, h * 512:(h + 1) * 512],
                             start=True, stop=True)
            nc.scalar.activation(GrowSb[0:KH, h * 512:(h + 1) * 512],
                                 psr[0:KH, :], Exp, scale=0.8)
            nc.scalar.copy(GrowSb[32:32 + KH, h * 512:(h + 1) * 512],
                           psr[0:KH, :])
        nc.sync.dma_start(gscr_d[:], GrowSb[:])
        Gball = big.tile([P, KH, NI], bf16, tag="Gball")
        Sball = big.tile([P, KH, NI], bf16, tag="Sball")
        for k in range(KH):
            nc.sync.dma_start(Gball[:, k, :], gscr_d[k, :].partition_broadcast(P))
        nc.sync.dma_start(Sball[:, 3, :], gscr_d[35, :].partition_broadcast(P))
        for k in range(KH - 1):
            nc.sync.dma_start(Sball[:, k, :],
                              gscr_d[32 + k, :].partition_broadcast(P))
        for jt in range(2, JT):  # bulk of the mask after the broadcasts
            nc.sync.dma_start(maskT[:, jt, :], mT_d[jt * P:(jt + 1) * P, :])

        # t-scores for all j: ptt[j, jt*4+k]; HF = exp t, F2 = exp 0.2t
        ptt = pst.tile([P, JT * KH], f32, tag="ptt")
        for jt in range(JT):
            nc.tensor.matmul(ptt[:, jt * KH:(jt + 1) * KH],
                             HT[:, jt * P:(jt + 1) * P], wsdst,
                             start=True, stop=True, skip_group_check=True)
        HFcol = big.tile([P, JT * KH], f32, tag="HFcol")
        F2col = big.tile([P, JT * KH], f32, tag="F2col")
        tcol = big.tile([P, JT * KH], f32, tag="tcol")
        t02col = big.tile([P, JT * KH], f32, tag="t02col")
        nc.scalar.activation(HFcol[:], ptt[:], Exp, scale=1.0)
        nc.scalar.activation(F2col[:], ptt[:], Exp, scale=0.2)
        nc.scalar.copy(tcol[:], ptt[:])
        nc.scalar.activation(t02col[:], ptt[:], Copy, scale=0.2)

        # ---- WhT = (H @ W).T bf16 (only feeds the PV stationaries) ----
        WhT = big.tile([P, N], bf16, tag="WhT")
        for q in range(N // 512):
            pw = ps.tile([P, 512], f32, tag="stg")
            nc.tensor.matmul(pw[:], sbW, HT[:, q * 512:(q + 1) * 512],
                             start=True, stop=True)
            nc.vector.tensor_copy(WhT[:, q * 512:(q + 1) * 512], pw[:])

        # whf[jt] = [Wh | 1] per head, bf16, via PE transpose (8 psum slots)
        pn8 = pst.tile([P, 8, P], bf16, tag="pn8")
        whf = []
        for jt in range(JT):
            nc.tensor.transpose(pn8[:, jt % 8, :], WhT[:, jt * P:(jt + 1) * P],
                                identb)
            wt = big.tile([P, KH, DH + 1], bf16, tag=f"whf{jt}", name=f"whf{jt}")
            nc.gpsimd.memset(wt[:, :, DH:DH + 1], 1.0)
            nc.scalar.copy(
                wt[:, :, 0:DH],
                pn8[:, jt % 8, :].rearrange("p (k d) -> p k d", k=KH),
            )
            whf.append(wt)

        # ---- main loop over j-chunks ----
        pv = [pspv.tile([97, 512], f32, tag=f"pv{q}", name=f"pv{q}")
              for q in range(4)]

        for jt in range(JT):
            v = vwork.tile([P, KH, NI], bf16, tag="v")
            for k in range(KH):
                e = PASS1_ENG[(jt, k)]
                c = jt * KH + k
                if e == "a":
                    rt = small.tile([P, NI], bf16, tag="rt", bufs=3)
                    nc.scalar.activation(rt[:], Sball[:, k, :], Relu,
                                         bias=tcol[:, c:c + 1], scale=1.0)
                    nc.scalar.activation(v[:, k, :], rt[:], Exp,
                                         bias=t02col[:, c:c + 1], scale=0.8)
                else:
                    eng = nc.vector if e == "v" else nc.gpsimd
                    eng.tensor_scalar(v[:, k, :], Gball[:, k, :],
                                      HFcol[:, c:c + 1], F2col[:, c:c + 1],
                                      MULT, MAX)
            y8 = ywork.tile([P, KH, NI], bf16, tag="y8")
            if jt in TT_POOL_JT:
                nc.vector.tensor_mul(
                    y8[:, 0:3], v[:, 0:3],
                    maskT[:, jt, None, :].broadcast_to((P, 3, NI)),
                )
                nc.gpsimd.tensor_mul(y8[:, 3], v[:, 3], maskT[:, jt, :])
            else:
                nc.vector.tensor_mul(
                    y8[:], v[:],
                    maskT[:, jt, None, :].broadcast_to((P, KH, NI)),
                )
            for ib in range(2):
                isl = slice(ib * 512, (ib + 1) * 512)
                for k in range(KH):
                    q, r = divmod(k, 2)
                    nc.tensor.matmul(
                        pv[q * 2 + ib][r * 64:r * 64 + 33, :],
                        whf[jt][:, k, :],
                        y8[:, k, isl],
                        start=(jt == 0), stop=(jt == JT - 1),
                        skip_group_check=True,
                    )

        # ---- epilogue: raw accumulators out; host divides / transposes ----
        for q in range(4):
            ot = small.tile([97, 512], f32, tag="ot")
            if q % 2 == 0:
                nc.vector.tensor_copy(ot[:], pv[q][:])
            else:
                nc.scalar.copy(ot[:], pv[q][:])
            nc.sync.dma_start(oaux_d[q, 0], ot[0:33, :])
            nc.sync.dma_start(oaux_d[q, 1], ot[64:97, :])

    nc.compile()
    return nc


def _host_prep(H, A, W, a_src, a_dst):
    """Build the 8 per-core input maps (layout prep + dtype casts only)."""
    Ssrc = np.zeros((FIN, KH), np.float32)
    Sdst = np.zeros((FIN, KH), np.float32)
    for k in range(KH):
        Ssrc[k * DH:(k + 1) * DH, k] = a_src[k]
        Sdst[k * DH:(k + 1) * DH, k] = a_dst[k]
    Wf = W.astype(np.float32)
    WSsrc = Wf @ Ssrc  # [FIN, KH]: s = H @ WSsrc
    WSdst = Wf @ Sdst

    wu_host = np.ones((P, WUC), np.float32)
    wu_host[:, 8:8 + P] = np.eye(P, dtype=np.float32)
    wu_host = wu_host.astype(BF)

    in_maps = []
    for c in range(8):
        b, half = divmod(c, 2)
        i0 = half * NI
        HbT = np.roll(H[b], -i0, axis=0).T  # [FIN, N], j rolled
        maskT = np.ascontiguousarray(
            (np.roll(A[b, i0:i0 + NI, :], -i0, axis=1) > 0).T
        ).astype(BF)
        cpack = np.concatenate(
            [Wf, WSsrc, WSdst, np.eye(P, dtype=np.float32), HbT],
            axis=1,
        ).astype(BF)
        in_maps.append({
            "wu": wu_host,
            "cpack": np.ascontiguousarray(cpack),
            "maskT": maskT,
        })
    return in_maps


def kernel(H, A, W, a_src, a_dst, _want_results=False, _trace=False):
    H = np.asarray(H); A = np.asarray(A); W = np.asarray(W)
    a_src = np.asarray(a_src); a_dst = np.asarray(a_dst)

    if "nc" not in _CACHE:
        _CACHE["nc"] = _build_program()
    nc = _CACHE["nc"]

    in_maps = _host_prep(H, A, W, a_src, a_dst)
    res = run_bass_kernel_spmd(nc, in_maps, list(range(8)), trace=_trace)

    out = np.empty((B, N, KH * DH), np.float32)
    for c in range(8):
        b, half = divmod(c, 2)
        i0 = half * NI
        aux = res.results[c]["oaux"]  # [4, 2, 33, 512] f32
        for q in range(4):
            p, ibb = divmod(q, 2)
            r0 = i0 + ibb * 512
            for h2 in range(2):
                k = 2 * p + h2
                blk = aux[q, h2]  # [33, 512]
                out[b, r0:r0 + 512, k * DH:(k + 1) * DH] = (
                    blk[0:DH] / blk[DH:DH + 1]
                ).T
    if _want_results:
        return out, res
    return out


# revision 10
# speedup vs baseline: 1.9397x; 1.0510x over previous
"""Trainium2 Bass kernel for a dense GAT layer (B=4, N=2048, FIN=128, K=4 heads, D=32).

Math (per batch b):
    Wh = (H @ W).reshape(N, K, D)
    s[i,k] = <Wh[i,k,:], a_src[k,:]>;  t[j,k] = <Wh[j,k,:], a_dst[k,:]>
    e[i,j,k] = leaky_relu(s[i,k] + t[j,k], 0.2), masked to -inf where A[i,j] == 0
    alpha = softmax_j(e);  out[i] = sum_j alpha[i,j,k] * Wh[j,k,:]

Reformulation (exact): with x = s_i + t_j,
    exp(lrelu(x)) = max(exp x, exp 0.2x); the i-side factor exp(0.2 s_i)
    cancels in the softmax.  With G = exp(0.8 s_i), HF_j = exp(t_j),
    F2_j = exp(0.2 t_j), m = (A > 0):
        v[j,i,k] = max(G_ik * HF_jk, F2_jk)          (= F2 * max(GH, 1))
        y[j,i,k] = v * m[j,i]
        num[i,k,:] = sum_j y * Wh[j,k,:];  den[i,k] = sum_j y
        out = num / den
    F2 folds into the tensor-scalar pass (HF as multiplier, F2 as max
    floor), so the PV stationary is raw [Wh | 1] and the denominator falls
    out of the ones column.  s and t come straight from H via host-side
    W@a_src / W@a_dst folds, so the score chain never waits on Wh.

All matmul operands and the score volume are bf16 (PE 1 cyc/row instead of
fp32's 4; DVE 4x mode on the tensor-scalar pass, 2x on the mask multiply).
The mask ships from the host as bf16 {0,1}.  Engine split per (jt, head) is
table-driven: DVE owns the mask TensorTensor (dominant pass) plus cheap 4x
TensorScalars, Pool takes one head's TensorScalar and some mask slices,
ACT absorbs one head via a relu+exp chain on broadcast s.

Sharding: 8 cores = 4 batches x 2 row-halves (i-slabs of 1024); no
collectives.  Host rotates H rows / A columns so each core's query rows are
local 0..1023 (identical SPMD program) and ships H pre-transposed.
"""

import numpy as np
import ml_dtypes
from contextlib import ExitStack

import concourse.bacc as bacc
import concourse.mybir as mybir
import concourse.tile as tile
from concourse.bass_utils import run_bass_kernel_spmd

B, N, FIN = 4, 2048, 128
KH, DH = 4, 32
P = 128
NI = 1024           # query rows per core
JT = N // P         # 16 j-chunks
WUC = 8 + 512       # warmup tile cols

f32 = mybir.dt.float32
bf16 = mybir.dt.bfloat16
BF = ml_dtypes.bfloat16

_CACHE = {}

# pass1 engine per (jt, k): 'v' = DVE TS, 'p' = Pool TS, 'a' = ACT relu+exp
PASS1_ENG = {}
for _jt in range(JT):
    PASS1_ENG[(_jt, 0)] = "v"
    PASS1_ENG[(_jt, 1)] = "p" if _jt % 3 == 0 else "v"
    PASS1_ENG[(_jt, 2)] = "p"
    PASS1_ENG[(_jt, 3)] = "a"
# jts whose k3 slice of the mask TT runs on Pool instead of DVE
TT_POOL_JT = {1, 4, 7, 10, 13}

JW0 = 5   # initial warmup junk matmuls


def _build_program():
    nc = bacc.Bacc("TRN2", target_bir_lowering=False, debug=False)

    def din(name, shape, dtype):
        return nc.dram_tensor(name, list(shape), dtype, kind="ExternalInput").ap()

    wu_d = din("wu", (P, WUC), bf16)      # tiny first DMA: PE warmup fodder
    CPW = P + 2 * KH + P + N              # [W | WSsrc | WSdst | identb | HT]
    cpack_d = din("cpack", (P, CPW), bf16)
    mT_d = din("maskT", (N, NI), bf16)    # mask (A>0) transposed: [j, i]
    gscr_d = nc.dram_tensor("gscr", [36, NI], bf16).ap()  # [Grow; pad; Sraw]
    oaux_d = nc.dram_tensor("oaux", [4, 2, 33, 512], f32,
                            kind="ExternalOutput").ap()

    Exp = mybir.ActivationFunctionType.Exp
    Relu = mybir.ActivationFunctionType.Relu
    Copy = mybir.ActivationFunctionType.Copy
    MULT = mybir.AluOpType.mult
    MAX = mybir.AluOpType.max

    with tile.TileContext(nc) as tc, ExitStack() as ctx:
        const = ctx.enter_context(tc.tile_pool(name="const", bufs=1))
        big = ctx.enter_context(tc.tile_pool(name="big", bufs=1))
        vwork = ctx.enter_context(tc.tile_pool(name="vwork", bufs=5))
        ywork = ctx.enter_context(tc.tile_pool(name="ywork", bufs=4))
        small = ctx.enter_context(tc.tile_pool(name="small", bufs=2))
        ps = ctx.enter_context(tc.tile_pool(name="ps", bufs=1, space="PSUM"))
        pst = ctx.enter_context(tc.tile_pool(name="pst", bufs=1, space="PSUM"))
        pspv = ctx.enter_context(tc.tile_pool(name="pspv", bufs=1, space="PSUM"))

        # ---- inputs; DMA queue order is the critical schedule ----
        wu = const.tile([P, WUC], bf16, tag="wu")
        nc.sync.dma_start(wu[:], wu_d[:])
        cpack = const.tile([P, CPW], bf16, tag="cpack")
        nc.sync.dma_start(cpack[:], cpack_d[:])
        sbW = cpack[:, 0:P]
        wssrc = cpack[:, P:P + KH]
        wsdst = cpack[:, P + KH:P + 2 * KH]
        identb = cpack[:, P + 2 * KH:2 * P + 2 * KH]
        HT = cpack[:, 2 * P + 2 * KH:]  # [fin, n]

        maskT = big.tile([P, JT, NI], bf16, tag="maskT")
        for jt in range(2):  # first two mask chunks up front
            nc.sync.dma_start(maskT[:, jt, :], mT_d[jt * P:(jt + 1) * P, :])

        # PE warm-up junk while cpack lands
        for _ in range(JW0):
            pj = ps.tile([P, 512], f32, tag="stg")
            nc.tensor.matmul(pj[0:8, :], wu[:, 0:8], wu[:, 8:8 + 512],
                             start=True, stop=True)

        # ---- srow/t straight from HT (host folded W into a_src/a_dst) ----
        GrowSb = small.tile([36, NI], bf16, tag="GrowSb", bufs=1)
        for h in range(2):
            psr = ps.tile([P, 512], f32, tag="stg")
            nc.tensor.matmul(psr[0:KH, :], wssrc,
                             HT[:, h * 512:(h + 1) * 512],
                             start=True, stop=True)
            nc.scalar.activation(GrowSb[0:KH, h * 512:(h + 1) * 512],
                                 psr[0:KH, :], Exp, scale=0.8)
            nc.scalar.copy(GrowSb[32:32 + KH, h * 512:(h + 1) * 512],
                           psr[0:KH, :])
        nc.sync.dma_start(gscr_d[:], GrowSb[:])
        Gball = big.tile([P, KH, NI], bf16, tag="Gball")
        Sball = big.tile([P, KH, NI], bf16, tag="Sball")
        for k in range(KH):
            nc.sync.dma_start(Gball[:, k, :], gscr_d[k, :].partition_broadcast(P))
        nc.sync.dma_start(Sball[:, 3, :], gscr_d[35, :].partition_broadcast(P))
        for k in range(KH - 1):
            nc.sync.dma_start(Sball[:, k, :],
                              gscr_d[32 + k, :].partition_broadcast(P))
        for jt in range(2, JT):  # bulk of the mask after the broadcasts
            nc.sync.dma_start(maskT[:, jt, :], mT_d[jt * P:(jt + 1) * P, :])

        # t-scores for all j: ptt[j, jt*4+k]; HF = exp t, F2 = exp 0.2t
        ptt = pst.tile([P, JT * KH], f32, tag="ptt")
        for jt in range(JT):
            nc.tensor.matmul(ptt[:, jt * KH:(jt + 1) * KH],
                             HT[:, jt * P:(jt + 1) * P], wsdst,
                             start=True, stop=True, skip_group_check=True)
        HFcol = big.tile([P, JT * KH], f32, tag="HFcol")
        F2col = big.tile([P, JT * KH], f32, tag="F2col")
        tcol = big.tile([P, JT * KH], f32, tag="tcol")
        t02col = big.tile([P, JT * KH], f32, tag="t02col")
        nc.scalar.activation(HFcol[:], ptt[:], Exp, scale=1.0)
        nc.scalar.activation(F2col[:], ptt[:], Exp, scale=0.2)
        nc.scalar.copy(tcol[:], ptt[:])
        nc.scalar.activation(t02col[:], ptt[:], Copy, scale=0.2)

        # ---- whf[jt] = [Wh | 1] per head: Wh computed directly in [j, kd]
        # layout (stationary = HT chunk, moving = W); copies PSUM->SBUF are
        # split DVE/ACT; jts >= 4 are emitted inside the main loop.
        pn8 = pst.tile([P, 6, P], f32, tag="pn8")
        whf = []

        def emit_whf(jt):
            nc.tensor.matmul(pn8[:, jt % 6, :], HT[:, jt * P:(jt + 1) * P],
                             sbW, start=True, stop=True)
            wt = big.tile([P, KH, DH + 1], bf16, tag=f"whf{jt}", name=f"whf{jt}")
            nc.gpsimd.memset(wt[:, :, DH:DH + 1], 1.0)
            dst = wt[:, :, 0:DH]
            srcv = pn8[:, jt % 6, :].rearrange("p (k d) -> p k d", k=KH)
            if jt % 2 == 0:
                nc.vector.tensor_copy(dst, srcv)
            else:
                nc.scalar.copy(dst, srcv)
            whf.append(wt)

        for jt in range(4):
            emit_whf(jt)

        # ---- main loop over j-chunks ----
        pv = [pspv.tile([97, 512], f32, tag=f"pv{q}", name=f"pv{q}")
              for q in range(4)]

        for jt in range(JT):
            if jt + 4 < JT:
                emit_whf(jt + 4)
            v = vwork.tile([P, KH, NI], bf16, tag="v")
            for k in range(KH):
                e = PASS1_ENG[(jt, k)]
                c = jt * KH + k
                if e == "a":
                    rt = small.tile([P, NI], bf16, tag="rt", bufs=3)
                    nc.scalar.activation(rt[:], Sball[:, k, :], Relu,
                                         bias=tcol[:, c:c + 1], scale=1.0)
                    nc.scalar.activation(v[:, k, :], rt[:], Exp,
                                         bias=t02col[:, c:c + 1], scale=0.8)
                else:
                    eng = nc.vector if e == "v" else nc.gpsimd
                    eng.tensor_scalar(v[:, k, :], Gball[:, k, :],
                                      HFcol[:, c:c + 1], F2col[:, c:c + 1],
                                      MULT, MAX)
            y8 = ywork.tile([P, KH, NI], bf16, tag="y8")
            if jt in TT_POOL_JT:
                nc.vector.tensor_mul(
                    y8[:, 0:3], v[:, 0:3],
                    maskT[:, jt, None, :].broadcast_to((P, 3, NI)),
                )
                nc.gpsimd.tensor_mul(y8[:, 3], v[:, 3], maskT[:, jt, :])
            else:
                nc.vector.tensor_mul(
                    y8[:], v[:],
                    maskT[:, jt, None, :].broadcast_to((P, KH, NI)),
                )
            for ib in range(2):
                isl = slice(ib * 512, (ib + 1) * 512)
                for k in range(KH):
                    q, r = divmod(k, 2)
                    nc.tensor.matmul(
                        pv[q * 2 + ib][r * 64:r * 64 + 33, :],
                        whf[jt][:, k, :],
                        y8[:, k, isl],
                        start=(jt == 0), stop=(jt == JT - 1),
                        skip_group_check=True,
                    )

        # ---- epilogue: raw accumulators out; host divides / transposes ----
        for q in range(4):
            ot = small.tile([97, 512], f32, tag="ot", bufs=4)
            if q % 2 == 0:
                nc.vector.tensor_copy(ot[:], pv[q][:])
            else:
                nc.scalar.copy(ot[:], pv[q][:])
            nc.sync.dma_start(oaux_d[q, 0], ot[0:33, :])
            nc.sync.dma_start(oaux_d[q, 1], ot[64:97, :])

    nc.compile()
    return nc


def _host_prep(H, A, W, a_src, a_dst):
    """Build the 8 per-core input maps (layout prep + dtype casts only)."""
    Ssrc = np.zeros((FIN, KH), np.float32)
    Sdst = np.zeros((FIN, KH), np.float32)
    for k in range(KH):
        Ssrc[k * DH:(k + 1) * DH, k] = a_src[k]
        Sdst[k * DH:(k + 1) * DH, k] = a_dst[k]
    Wf = W.astype(np.float32)
    WSsrc = Wf @ Ssrc  # [FIN, KH]: s = H @ WSsrc
    WSdst = Wf @ Sdst

    wu_host = np.ones((P, WUC), np.float32)
    wu_host[:, 8:8 + P] = np.eye(P, dtype=np.float32)
    wu_host = wu_host.astype(BF)

    in_maps = []
    for c in range(8):
        b, half = divmod(c, 2)
        i0 = half * NI
        HbT = np.roll(H[b], -i0, axis=0).T  # [FIN, N], j rolled
        maskT = np.ascontiguousarray(
            (np.roll(A[b, i0:i0 + NI, :], -i0, axis=1) > 0).T
        ).astype(BF)
        cpack = np.concatenate(
            [Wf, WSsrc, WSdst, np.eye(P, dtype=np.float32), HbT],
            axis=1,
        ).astype(BF)
        in_maps.append({
            "wu": wu_host,
            "cpack": np.ascontiguousarray(cpack),
            "maskT": maskT,
        })
    return in_maps


def kernel(H, A, W, a_src, a_dst, _want_results=False, _trace=False):
    H = np.asarray(H); A = np.asarray(A); W = np.asarray(W)
    a_src = np.asarray(a_src); a_dst = np.asarray(a_dst)

    if "nc" not in _CACHE:
        _CACHE["nc"] = _build_program()
    nc = _CACHE["nc"]

    in_maps = _host_prep(H, A, W, a_src, a_dst)
    res = run_bass_kernel_spmd(nc, in_maps, list(range(8)), trace=_trace)

    out = np.empty((B, N, KH * DH), np.float32)
    for c in range(8):
        b, half = divmod(c, 2)
        i0 = half * NI
        aux = res.results[c]["oaux"]  # [4, 2, 33, 512] f32
        for q in range(4):
            p, ibb = divmod(q, 2)
            r0 = i0 + ibb * 512
            for h2 in range(2):
                k = 2 * p + h2
                blk = aux[q, h2]  # [33, 512]
                out[b, r0:r0 + 512, k * DH:(k + 1) * DH] = (
                    blk[0:DH] / blk[DH:DH + 1]
                ).T
    if _want_results:
        return out, res
    return out


# revision 11
# speedup vs baseline: 1.9466x; 1.0036x over previous
"""Trainium2 Bass kernel for a dense GAT layer (B=4, N=2048, FIN=128, K=4 heads, D=32).

Math (per batch b):
    Wh = (H @ W).reshape(N, K, D)
    s[i,k] = <Wh[i,k,:], a_src[k,:]>;  t[j,k] = <Wh[j,k,:], a_dst[k,:]>
    e[i,j,k] = leaky_relu(s[i,k] + t[j,k], 0.2), masked to -inf where A[i,j] == 0
    alpha = softmax_j(e);  out[i] = sum_j alpha[i,j,k] * Wh[j,k,:]

Reformulation (exact): with x = s_i + t_j,
    exp(lrelu(x)) = max(exp x, exp 0.2x); the i-side factor exp(0.2 s_i)
    cancels in the softmax.  With G = exp(0.8 s_i), HF_j = exp(t_j),
    F2_j = exp(0.2 t_j), m = (A > 0):
        v[j,i,k] = max(G_ik * HF_jk, F2_jk)          (= F2 * max(GH, 1))
        y[j,i,k] = v * m[j,i]
        num[i,k,:] = sum_j y * Wh[j,k,:];  den[i,k] = sum_j y
        out = num / den
    F2 folds into the tensor-scalar pass (HF as multiplier, F2 as max
    floor), so the PV stationary is raw [Wh | 1] and the denominator falls
    out of the ones column.  s and t come straight from H via host-side
    W@a_src / W@a_dst folds, so the score chain never waits on Wh.

All matmul operands and the score volume are bf16 (PE 1 cyc/row instead of
fp32's 4; DVE 4x mode on the tensor-scalar pass, 2x on the mask multiply).
The mask ships from the host as bf16 {0,1}.  Engine split per (jt, head) is
table-driven: DVE owns the mask TensorTensor (dominant pass) plus cheap 4x
TensorScalars, Pool takes one head's TensorScalar and some mask slices,
ACT absorbs one head via a relu+exp chain on broadcast s.

Sharding: 8 cores = 4 batches x 2 row-halves (i-slabs of 1024); no
collectives.  Host rotates H rows / A columns so each core's query rows are
local 0..1023 (identical SPMD program) and ships H pre-transposed.
"""

import numpy as np
import ml_dtypes
from contextlib import ExitStack

import concourse.bacc as bacc
import concourse.mybir as mybir
import concourse.tile as tile
from concourse.bass_utils import run_bass_kernel_spmd

B, N, FIN = 4, 2048, 128
KH, DH = 4, 32
P = 128
NI = 1024           # query rows per core
JT = N // P         # 16 j-chunks
WUC = 8 + 512       # warmup tile cols

f32 = mybir.dt.float32
bf16 = mybir.dt.bfloat16
BF = ml_dtypes.bfloat16

_CACHE = {}

# pass1 engine per (jt, k): 'v' = DVE TS, 'p' = Pool TS, 'a' = ACT relu+exp
PASS1_ENG = {}
for _jt in range(JT):
    PASS1_ENG[(_jt, 0)] = "v"
    PASS1_ENG[(_jt, 1)] = "p" if _jt % 4 == 0 else "v"
    PASS1_ENG[(_jt, 2)] = "p"
    PASS1_ENG[(_jt, 3)] = "v" if _jt < 3 else "a"
# jts whose k3 slice of the mask TT runs on Pool instead of DVE
TT_POOL_JT = {1, 4, 7, 10, 13}

JW0 = 5   # initial warmup junk matmuls


def _build_program():
    nc = bacc.Bacc("TRN2", target_bir_lowering=False, debug=False)

    def din(name, shape, dtype):
        return nc.dram_tensor(name, list(shape), dtype, kind="ExternalInput").ap()

    wu_d = din("wu", (P, WUC), bf16)      # tiny first DMA: PE warmup fodder
    CPW = P + 2 * KH + P + N              # [W | WSsrc | WSdst | identb | HT]
    cpack_d = din("cpack", (P, CPW), bf16)
    CP0 = 2 * P + 2 * KH                  # small head of cpack (no HT)
    mT_d = din("maskT", (N, NI), bf16)    # mask (A>0) transposed: [j, i]
    gscr_d = nc.dram_tensor("gscr", [36, NI], bf16).ap()  # [Grow; pad; Sraw]
    oaux_d = nc.dram_tensor("oaux", [4, 2, 33, 512], f32,
                            kind="ExternalOutput").ap()

    Exp = mybir.ActivationFunctionType.Exp
    Relu = mybir.ActivationFunctionType.Relu
    Copy = mybir.ActivationFunctionType.Copy
    MULT = mybir.AluOpType.mult
    MAX = mybir.AluOpType.max

    with tile.TileContext(nc) as tc, ExitStack() as ctx:
        const = ctx.enter_context(tc.tile_pool(name="const", bufs=1))
        big = ctx.enter_context(tc.tile_pool(name="big", bufs=1))
        vwork = ctx.enter_context(tc.tile_pool(name="vwork", bufs=5))
        ywork = ctx.enter_context(tc.tile_pool(name="ywork", bufs=4))
        small = ctx.enter_context(tc.tile_pool(name="small", bufs=2))
        ps = ctx.enter_context(tc.tile_pool(name="ps", bufs=1, space="PSUM"))
        pst = ctx.enter_context(tc.tile_pool(name="pst", bufs=1, space="PSUM"))
        pspv = ctx.enter_context(tc.tile_pool(name="pspv", bufs=1, space="PSUM"))

        # ---- inputs; DMA queue order is the critical schedule ----
        wu = const.tile([P, WUC], bf16, tag="wu")
        nc.sync.dma_start(wu[:], wu_d[:])
        cpack = const.tile([P, CPW], bf16, tag="cpack")
        nc.sync.dma_start(cpack[:, 0:CP0], cpack_d[:, 0:CP0])
        nc.sync.dma_start(cpack[:, CP0:], cpack_d[:, CP0:])
        sbW = cpack[:, 0:P]
        wssrc = cpack[:, P:P + KH]
        wsdst = cpack[:, P + KH:P + 2 * KH]
        identb = cpack[:, P + 2 * KH:2 * P + 2 * KH]
        HT = cpack[:, 2 * P + 2 * KH:]  # [fin, n]

        maskT = big.tile([P, JT, NI], bf16, tag="maskT")
        for jt in range(2):  # first two mask chunks up front
            nc.sync.dma_start(maskT[:, jt, :], mT_d[jt * P:(jt + 1) * P, :])

        # PE warm-up junk while cpack lands
        for _ in range(JW0):
            pj = ps.tile([P, 512], f32, tag="stg")
            nc.tensor.matmul(pj[0:8, :], wu[:, 0:8], wu[:, 8:8 + 512],
                             start=True, stop=True)

        # ---- srow/t straight from HT (host folded W into a_src/a_dst) ----
        GrowSb = small.tile([36, NI], bf16, tag="GrowSb", bufs=1)
        for h in range(2):
            psr = ps.tile([P, 512], f32, tag="stg")
            nc.tensor.matmul(psr[0:KH, :], wssrc,
                             HT[:, h * 512:(h + 1) * 512],
                             start=True, stop=True)
            nc.scalar.activation(GrowSb[0:KH, h * 512:(h + 1) * 512],
                                 psr[0:KH, :], Exp, scale=0.8)
            nc.scalar.copy(GrowSb[32:32 + KH, h * 512:(h + 1) * 512],
                           psr[0:KH, :])
        nc.sync.dma_start(gscr_d[:], GrowSb[:])
        Gball = big.tile([P, KH, NI], bf16, tag="Gball")
        Sball = big.tile([P, 1, NI], bf16, tag="Sball")
        for k in [0, 3, 1, 2]:
            nc.sync.dma_start(Gball[:, k, :], gscr_d[k, :].partition_broadcast(P))
        nc.sync.dma_start(Sball[:, 0, :], gscr_d[35, :].partition_broadcast(P))
        for jt in range(2, JT):  # bulk of the mask after the broadcasts
            nc.sync.dma_start(maskT[:, jt, :], mT_d[jt * P:(jt + 1) * P, :])

        # t-scores for all j: ptt[j, jt*4+k]; HF = exp t, F2 = exp 0.2t
        ptt = pst.tile([P, JT * KH], f32, tag="ptt")
        for jt in range(JT):
            nc.tensor.matmul(ptt[:, jt * KH:(jt + 1) * KH],
                             HT[:, jt * P:(jt + 1) * P], wsdst,
                             start=True, stop=True, skip_group_check=True)
        HFcol = big.tile([P, JT * KH], f32, tag="HFcol")
        F2col = big.tile([P, JT * KH], f32, tag="F2col")
        tcol = big.tile([P, JT * KH], f32, tag="tcol")
        t02col = big.tile([P, JT * KH], f32, tag="t02col")
        nc.scalar.activation(HFcol[:], ptt[:], Exp, scale=1.0)
        nc.scalar.activation(F2col[:], ptt[:], Exp, scale=0.2)
        nc.scalar.copy(tcol[:], ptt[:])
        nc.scalar.activation(t02col[:], ptt[:], Copy, scale=0.2)

        # ---- whf[jt] = [Wh | 1] per head: Wh computed directly in [j, kd]
        # layout (stationary = HT chunk, moving = W); copies PSUM->SBUF are
        # split DVE/ACT; jts >= 4 are emitted inside the main loop.
        pn8 = pst.tile([P, 6, P], f32, tag="pn8")
        whf = []

        def emit_whf(jt):
            nc.tensor.matmul(pn8[:, jt % 6, :], HT[:, jt * P:(jt + 1) * P],
                             sbW, start=True, stop=True)
            wt = big.tile([P, KH, DH + 1], bf16, tag=f"whf{jt}", name=f"whf{jt}")
            nc.gpsimd.memset(wt[:, :, DH:DH + 1], 1.0)
            nc.scalar.copy(
                wt[:, :, 0:DH],
                pn8[:, jt % 6, :].rearrange("p (k d) -> p k d", k=KH),
            )
            whf.append(wt)

        for jt in range(4):
            emit_whf(jt)

        # ---- main loop over j-chunks ----
        pv = [pspv.tile([97, 512], f32, tag=f"pv{q}", name=f"pv{q}")
              for q in range(4)]

        for jt in range(JT):
            if jt + 4 < JT:
                emit_whf(jt + 4)
            v = vwork.tile([P, KH, NI], bf16, tag="v")
            for k in range(KH):
                e = PASS1_ENG[(jt, k)]
                c = jt * KH + k
                if e == "a":
                    rt = small.tile([P, NI], bf16, tag="rt", bufs=3)
                    nc.scalar.activation(rt[:], Sball[:, 0, :], Relu,
                                         bias=tcol[:, c:c + 1], scale=1.0)
                    nc.scalar.activation(v[:, k, :], rt[:], Exp,
                                         bias=t02col[:, c:c + 1], scale=0.8)
                else:
                    eng = nc.vector if e == "v" else nc.gpsimd
                    eng.tensor_scalar(v[:, k, :], Gball[:, k, :],
                                      HFcol[:, c:c + 1], F2col[:, c:c + 1],
                                      MULT, MAX)
            y8 = ywork.tile([P, KH, NI], bf16, tag="y8")
            if jt in TT_POOL_JT:
                nc.vector.tensor_mul(
                    y8[:, 0:3], v[:, 0:3],
                    maskT[:, jt, None, :].broadcast_to((P, 3, NI)),
                )
                nc.gpsimd.tensor_mul(y8[:, 3], v[:, 3], maskT[:, jt, :])
            else:
                nc.vector.tensor_mul(
                    y8[:], v[:],
                    maskT[:, jt, None, :].broadcast_to((P, KH, NI)),
                )
            for ib in range(2):
                isl = slice(ib * 512, (ib + 1) * 512)
                for k in range(KH):
                    q, r = divmod(k, 2)
                    nc.tensor.matmul(
                        pv[q * 2 + ib][r * 64:r * 64 + 33, :],
                        whf[jt][:, k, :],
                        y8[:, k, isl],
                        start=(jt == 0), stop=(jt == JT - 1),
                        skip_group_check=True,
                    )

        # ---- epilogue: raw accumulators out; host divides / transposes ----
        for q in range(4):
            ot = small.tile([97, 512], f32, tag="ot", bufs=4)
            if q % 2 == 0:
                nc.vector.tensor_copy(ot[:], pv[q][:])
            else:
                nc.scalar.copy(ot[:], pv[q][:])
            nc.sync.dma_start(oaux_d[q, 0], ot[0:33, :])
            nc.sync.dma_start(oaux_d[q, 1], ot[64:97, :])

    nc.compile()
    return nc


def _host_prep(H, A, W, a_src, a_dst):
    """Build the 8 per-core input maps (layout prep + dtype casts only)."""
    Ssrc = np.zeros((FIN, KH), np.float32)
    Sdst = np.zeros((FIN, KH), np.float32)
    for k in range(KH):
        Ssrc[k * DH:(k + 1) * DH, k] = a_src[k]
        Sdst[k * DH:(k + 1) * DH, k] = a_dst[k]
    Wf = W.astype(np.float32)
    WSsrc = Wf @ Ssrc  # [FIN, KH]: s = H @ WSsrc
    WSdst = Wf @ Sdst

    wu_host = np.ones((P, WUC), np.float32)
    wu_host[:, 8:8 + P] = np.eye(P, dtype=np.float32)
    wu_host = wu_host.astype(BF)

    in_maps = []
    for c in range(8):
        b, half = divmod(c, 2)
        i0 = half * NI
        HbT = np.roll(H[b], -i0, axis=0).T  # [FIN, N], j rolled
        maskT = np.ascontiguousarray(
            (np.roll(A[b, i0:i0 + NI, :], -i0, axis=1) > 0).T
        ).astype(BF)
        cpack = np.concatenate(
            [Wf, WSsrc, WSdst, np.eye(P, dtype=np.float32), HbT],
            axis=1,
        ).astype(BF)
        in_maps.append({
            "wu": wu_host,
            "cpack": np.ascontiguousarray(cpack),
            "maskT": maskT,
        })
    return in_maps


def kernel(H, A, W, a_src, a_dst, _want_results=False, _trace=False):
    H = np.asarray(H); A = np.asarray(A); W = np.asarray(W)
    a_src = np.asarray(a_src); a_dst = np.asarray(a_dst)

    if "nc" not in _CACHE:
        _CACHE["nc"] = _build_program()
    nc = _CACHE["nc"]

    in_maps = _host_prep(H, A, W, a_src, a_dst)
    res = run_bass_kernel_spmd(nc, in_maps, list(range(8)), trace=_trace)

    out = np.empty((B, N, KH * DH), np.float32)
    for c in range(8):
        b, half = divmod(c, 2)
        i0 = half * NI
        aux = res.results[c]["oaux"]  # [4, 2, 33, 512] f32
        for q in range(4):
            p, ibb = divmod(q, 2)
            r0 = i0 + ibb * 512
            for h2 in range(2):
                k = 2 * p + h2
                blk = aux[q, h2]  # [33, 512]
                out[b, r0:r0 + 512, k * DH:(k + 1) * DH] = (
                    blk[0:DH] / blk[DH:DH + 1]
                ).T
    if _want_results:
        return out, res
    return out


# revision 12
# speedup vs baseline: 1.9594x; 1.0066x over previous
"""Trainium2 Bass kernel for a dense GAT layer (B=4, N=2048, FIN=128, K=4 heads, D=32).

Math (per batch b):
    Wh = (H @ W).reshape(N, K, D)
    s[i,k] = <Wh[i,k,:], a_src[k,:]>;  t[j,k] = <Wh[j,k,:], a_dst[k,:]>
    e[i,j,k] = leaky_relu(s[i,k] + t[j,k], 0.2), masked to -inf where A[i,j] == 0
    alpha = softmax_j(e);  out[i] = sum_j alpha[i,j,k] * Wh[j,k,:]

Reformulation (exact): with x = s_i + t_j,
    exp(lrelu(x)) = max(exp x, exp 0.2x); the i-side factor exp(0.2 s_i)
    cancels in the softmax.  With G = exp(0.8 s_i), HF_j = exp(t_j),
    F2_j = exp(0.2 t_j), m = (A > 0):
        v[j,i,k] = max(G_ik * HF_jk, F2_jk)          (= F2 * max(GH, 1))
        y[j,i,k] = v * m[j,i]
        num[i,k,:] = sum_j y * Wh[j,k,:];  den[i,k] = sum_j y
        out = num / den
    F2 folds into the tensor-scalar pass (HF as multiplier, F2 as max
    floor), so the PV stationary is raw [Wh | 1] and the denominator falls
    out of the ones column.  s and t come straight from H via host-side
    W@a_src / W@a_dst folds, so the score chain never waits on Wh.

All matmul operands and the score volume are bf16 (PE 1 cyc/row instead of
fp32's 4; DVE 4x mode on the tensor-scalar pass, 2x on the mask multiply).
The mask ships from the host as bf16 {0,1}.  Engine split per (jt, head) is
table-driven: DVE owns the mask TensorTensor (dominant pass) plus cheap 4x
TensorScalars, Pool takes one head's TensorScalar and some mask slices,
ACT absorbs one head via a relu+exp chain on broadcast s.

Sharding: 8 cores = 4 batches x 2 row-halves (i-slabs of 1024); no
collectives.  Host rotates H rows / A columns so each core's query rows are
local 0..1023 (identical SPMD program) and ships H pre-transposed.
"""

import numpy as np
import ml_dtypes
from contextlib import ExitStack

import concourse.bacc as bacc
import concourse.mybir as mybir
import concourse.tile as tile
from concourse.bass_utils import run_bass_kernel_spmd

B, N, FIN = 4, 2048, 128
KH, DH = 4, 32
P = 128
NI = 1024           # query rows per core
JT = N // P         # 16 j-chunks
WUC = 8 + 512       # warmup tile cols

f32 = mybir.dt.float32
bf16 = mybir.dt.bfloat16
BF = ml_dtypes.bfloat16

_CACHE = {}

# pass1 engine per (jt, k): 'v' = DVE TS, 'p' = Pool TS, 'a' = ACT relu+exp
PASS1_ENG = {}
for _jt in range(JT):
    PASS1_ENG[(_jt, 0)] = "v"
    PASS1_ENG[(_jt, 1)] = "p" if _jt % 4 == 0 else "v"
    PASS1_ENG[(_jt, 2)] = "p"
    PASS1_ENG[(_jt, 3)] = "v" if _jt < 3 else "a"
# jts whose k3 slice of the mask TT runs on Pool instead of DVE
TT_POOL_JT = {1, 4, 7, 10, 13}

JW0 = 5   # initial warmup junk matmuls


def _build_program():
    nc = bacc.Bacc("TRN2", target_bir_lowering=False, debug=False)

    def din(name, shape, dtype):
        return nc.dram_tensor(name, list(shape), dtype, kind="ExternalInput").ap()

    wu_d = din("wu", (P, WUC), bf16)      # tiny first DMA: PE warmup fodder
    CPW = P + 2 * KH + P + N              # [W | WSsrc | WSdst | identb | HT]
    cpack_d = din("cpack", (P, CPW), bf16)
    CP0 = 2 * P + 2 * KH                  # small head of cpack (no HT)
    mT_d = din("maskT", (N, NI), bf16)    # mask (A>0) transposed: [j, i]
    gscr_d = nc.dram_tensor("gscr", [36, NI], bf16).ap()  # [Grow; pad; Sraw]
    oaux_d = nc.dram_tensor("oaux", [4, 2, 33, 512], f32,
                            kind="ExternalOutput").ap()

    Exp = mybir.ActivationFunctionType.Exp
    Relu = mybir.ActivationFunctionType.Relu
    Copy = mybir.ActivationFunctionType.Copy
    MULT = mybir.AluOpType.mult
    MAX = mybir.AluOpType.max

    with tile.TileContext(nc) as tc, ExitStack() as ctx:
        const = ctx.enter_context(tc.tile_pool(name="const", bufs=1))
        big = ctx.enter_context(tc.tile_pool(name="big", bufs=1))
        vwork = ctx.enter_context(tc.tile_pool(name="vwork", bufs=5))
        ywork = ctx.enter_context(tc.tile_pool(name="ywork", bufs=4))
        small = ctx.enter_context(tc.tile_pool(name="small", bufs=2))
        ps = ctx.enter_context(tc.tile_pool(name="ps", bufs=1, space="PSUM"))
        pst = ctx.enter_context(tc.tile_pool(name="pst", bufs=1, space="PSUM"))
        pspv = ctx.enter_context(tc.tile_pool(name="pspv", bufs=1, space="PSUM"))

        # ---- inputs; DMA queue order is the critical schedule ----
        wu = const.tile([P, WUC], bf16, tag="wu")
        nc.sync.dma_start(wu[:], wu_d[:])
        cpack = const.tile([P, CPW], bf16, tag="cpack")
        nc.sync.dma_start(cpack[:, 0:CP0], cpack_d[:, 0:CP0])
        nc.sync.dma_start(cpack[:, CP0:], cpack_d[:, CP0:])
        sbW = cpack[:, 0:P]
        wssrc = cpack[:, P:P + KH]
        wsdst = cpack[:, P + KH:P + 2 * KH]
        identb = cpack[:, P + 2 * KH:2 * P + 2 * KH]
        HT = cpack[:, 2 * P + 2 * KH:]  # [fin, n]

        maskT = big.tile([P, JT, NI], bf16, tag="maskT")
        for jt in range(2):  # first two mask chunks up front
            nc.sync.dma_start(maskT[:, jt, :], mT_d[jt * P:(jt + 1) * P, :])

        # PE warm-up junk while cpack lands
        for _ in range(JW0):
            pj = ps.tile([P, 512], f32, tag="stg")
            nc.tensor.matmul(pj[0:8, :], wu[:, 0:8], wu[:, 8:8 + 512],
                             start=True, stop=True)

        # ---- srow/t straight from HT (host folded W into a_src/a_dst) ----
        GrowSb = small.tile([36, NI], bf16, tag="GrowSb", bufs=1)
        for h in range(2):
            psr = ps.tile([P, 512], f32, tag="stg")
            nc.tensor.matmul(psr[0:KH, :], wssrc,
                             HT[:, h * 512:(h + 1) * 512],
                             start=True, stop=True)
            nc.scalar.activation(GrowSb[0:KH, h * 512:(h + 1) * 512],
                                 psr[0:KH, :], Exp, scale=0.8)
            nc.scalar.copy(GrowSb[32:32 + KH, h * 512:(h + 1) * 512],
                           psr[0:KH, :])
        nc.sync.dma_start(gscr_d[:], GrowSb[:])
        Gball = big.tile([P, KH, NI], bf16, tag="Gball")
        Sball = big.tile([P, 1, NI], bf16, tag="Sball")
        for k in [0, 3, 1, 2]:
            nc.sync.dma_start(Gball[:, k, :], gscr_d[k, :].partition_broadcast(P))
        nc.sync.dma_start(Sball[:, 0, :], gscr_d[35, :].partition_broadcast(P))
        for jt in range(2, 5):
            nc.sync.dma_start(maskT[:, jt, :], mT_d[jt * P:(jt + 1) * P, :])
        # hold the bulk of the mask until the broadcast chain has the DMA
        # engine to itself (readiness-based arbitration would front-run it)
        with tc.tile_wait_until(0.013):
            for jt in range(5, JT):
                nc.sync.dma_start(maskT[:, jt, :], mT_d[jt * P:(jt + 1) * P, :])

        # t-scores for all j: ptt[j, jt*4+k]; HF = exp t, F2 = exp 0.2t
        ptt = pst.tile([P, JT * KH], f32, tag="ptt")
        for jt in range(JT):
            nc.tensor.matmul(ptt[:, jt * KH:(jt + 1) * KH],
                             HT[:, jt * P:(jt + 1) * P], wsdst,
                             start=True, stop=True, skip_group_check=True)
        HFcol = big.tile([P, JT * KH], f32, tag="HFcol")
        F2col = big.tile([P, JT * KH], f32, tag="F2col")
        tcol = big.tile([P, JT * KH], f32, tag="tcol")
        t02col = big.tile([P, JT * KH], f32, tag="t02col")
        nc.scalar.activation(HFcol[:], ptt[:], Exp, scale=1.0)
        nc.scalar.activation(F2col[:], ptt[:], Exp, scale=0.2)
        nc.scalar.copy(tcol[:], ptt[:])
        nc.scalar.activation(t02col[:], ptt[:], Copy, scale=0.2)

        # ---- whf[jt] = [Wh | 1] per head: Wh computed directly in [j, kd]
        # layout (stationary = HT chunk, moving = W); copies PSUM->SBUF are
        # split DVE/ACT; jts >= 4 are emitted inside the main loop.
        pn8 = pst.tile([P, 6, P], f32, tag="pn8")
        whf = []

        def emit_whf(jt):
            nc.tensor.matmul(pn8[:, jt % 6, :], HT[:, jt * P:(jt + 1) * P],
                             sbW, start=True, stop=True)
            wt = big.tile([P, KH, DH + 1], bf16, tag=f"whf{jt}", name=f"whf{jt}")
            nc.gpsimd.memset(wt[:, :, DH:DH + 1], 1.0)
            nc.scalar.copy(
                wt[:, :, 0:DH],
                pn8[:, jt % 6, :].rearrange("p (k d) -> p k d", k=KH),
            )
            whf.append(wt)

        for jt in range(4):
            emit_whf(jt)

        # ---- main loop over j-chunks ----
        pv = [pspv.tile([97, 512], f32, tag=f"pv{q}", name=f"pv{q}")
              for q in range(4)]

        for jt in range(JT):
            if jt + 4 < JT:
                emit_whf(jt + 4)
            v = vwork.tile([P, KH, NI], bf16, tag="v")
            for k in range(KH):
                e = PASS1_ENG[(jt, k)]
                c = jt * KH + k
                if e == "a":
                    rt = small.tile([P, NI], bf16, tag="rt", bufs=3)
                    nc.scalar.activation(rt[:], Sball[:, 0, :], Relu,
                                         bias=tcol[:, c:c + 1], scale=1.0)
                    nc.scalar.activation(v[:, k, :], rt[:], Exp,
                                         bias=t02col[:, c:c + 1], scale=0.8)
                else:
                    eng = nc.vector if e == "v" else nc.gpsimd
                    eng.tensor_scalar(v[:, k, :], Gball[:, k, :],
                                      HFcol[:, c:c + 1], F2col[:, c:c + 1],
                                      MULT, MAX)
            y8 = ywork.tile([P, KH, NI], bf16, tag="y8")
            if jt in TT_POOL_JT:
                nc.vector.tensor_mul(
                    y8[:, 0:3], v[:, 0:3],
                    maskT[:, jt, None, :].broadcast_to((P, 3, NI)),
                )
                nc.gpsimd.tensor_mul(y8[:, 3], v[:, 3], maskT[:, jt, :])
            else:
                nc.vector.tensor_mul(
                    y8[:], v[:],
                    maskT[:, jt, None, :].broadcast_to((P, KH, NI)),
                )
            for ib in range(2):
                isl = slice(ib * 512, (ib + 1) * 512)
                for k in range(KH):
                    q, r = divmod(k, 2)
                    nc.tensor.matmul(
                        pv[q * 2 + ib][r * 64:r * 64 + 33, :],
                        whf[jt][:, k, :],
                        y8[:, k, isl],
                        start=(jt == 0), stop=(jt == JT - 1),
                        skip_group_check=True,
                    )

        # ---- epilogue: raw accumulators out; host divides / transposes ----
        for q in range(4):
            ot = small.tile([97, 512], f32, tag="ot", bufs=4)
            if q % 2 == 0:
                nc.vector.tensor_copy(ot[:], pv[q][:])
            else:
                nc.scalar.copy(ot[:], pv[q][:])
            nc.sync.dma_start(oaux_d[q, 0], ot[0:33, :])
            nc.sync.dma_start(oaux_d[q, 1], ot[64:97, :])

    nc.compile()
    return nc


def _host_prep(H, A, W, a_src, a_dst):
    """Build the 8 per-core input maps (layout prep + dtype casts only)."""
    Ssrc = np.zeros((FIN, KH), np.float32)
    Sdst = np.zeros((FIN, KH), np.float32)
    for k in range(KH):
        Ssrc[k * DH:(k + 1) * DH, k] = a_src[k]
        Sdst[k * DH:(k + 1) * DH, k] = a_dst[k]
    Wf = W.astype(np.float32)
    WSsrc = Wf @ Ssrc  # [FIN, KH]: s = H @ WSsrc
    WSdst = Wf @ Sdst

    wu_host = np.ones((P, WUC), np.float32)
    wu_host[:, 8:8 + P] = np.eye(P, dtype=np.float32)
    wu_host = wu_host.astype(BF)

    in_maps = []
    for c in range(8):
        b, half = divmod(c, 2)
        i0 = half * NI
        HbT = np.roll(H[b], -i0, axis=0).T  # [FIN, N], j rolled
        maskT = np.ascontiguousarray(
            (np.roll(A[b, i0:i0 + NI, :], -i0, axis=1) > 0).T
        ).astype(BF)
        cpack = np.concatenate(
            [Wf, WSsrc, WSdst, np.eye(P, dtype=np.float32), HbT],
            axis=1,
        ).astype(BF)
        in_maps.append({
            "wu": wu_host,
            "cpack": np.ascontiguousarray(cpack),
            "maskT": maskT,
        })
    return in_maps


def kernel(H, A, W, a_src, a_dst, _want_results=False, _trace=False):
    H = np.asarray(H); A = np.asarray(A); W = np.asarray(W)
    a_src = np.asarray(a_src); a_dst = np.asarray(a_dst)

    if "nc" not in _CACHE:
        _CACHE["nc"] = _build_program()
    nc = _CACHE["nc"]

    in_maps = _host_prep(H, A, W, a_src, a_dst)
    res = run_bass_kernel_spmd(nc, in_maps, list(range(8)), trace=_trace)

    out = np.empty((B, N, KH * DH), np.float32)
    for c in range(8):
        b, half = divmod(c, 2)
        i0 = half * NI
        aux = res.results[c]["oaux"]  # [4, 2, 33, 512] f32
        for q in range(4):
            p, ibb = divmod(q, 2)
            r0 = i0 + ibb * 512
            for h2 in range(2):
                k = 2 * p + h2
                blk = aux[q, h2]  # [33, 512]
                out[b, r0:r0 + 512, k * DH:(k + 1) * DH] = (
                    blk[0:DH] / blk[DH:DH + 1]
                ).T
    if _want_results:
        return out, res
    return out


# revision 15
# speedup vs baseline: 2.0117x; 1.0267x over previous
"""Trainium2 Bass kernel for a dense GAT layer (B=4, N=2048, FIN=128, K=4 heads, D=32).

Math (per batch b):
    Wh = (H @ W).reshape(N, K, D)
    s[i,k] = <Wh[i,k,:], a_src[k,:]>;  t[j,k] = <Wh[j,k,:], a_dst[k,:]>
    e[i,j,k] = leaky_relu(s[i,k] + t[j,k], 0.2), masked to -inf where A[i,j] == 0
    alpha = softmax_j(e);  out[i] = sum_j alpha[i,j,k] * Wh[j,k,:]

Reformulation (exact): with x = s_i + t_j,
    exp(lrelu(x)) = max(exp x, exp 0.2x); the i-side factor exp(0.2 s_i)
    cancels in the softmax.  With G = exp(0.8 s_i), HF_j = exp(t_j),
    F2_j = exp(0.2 t_j), m = (A > 0):
        v[j,i,k] = max(G_ik * HF_jk, F2_jk)          (= F2 * max(GH, 1))
        y[j,i,k] = v * m[j,i]
        num[i,k,:] = sum_j y * Wh[j,k,:];  den[i,k] = sum_j y
        out = num / den
    F2 folds into the tensor-scalar pass (HF as multiplier, F2 as max
    floor), so the PV stationary is raw [Wh | 1] and the denominator falls
    out of the ones column.  s and t come straight from H via host-side
    W@a_src / W@a_dst folds, so the score chain never waits on Wh.

All matmul operands and the score volume are bf16 (PE 1 cyc/row instead of
fp32's 4; DVE 4x mode on the tensor-scalar pass, 2x on the mask multiply).
The mask ships from the host as bf16 {0,1}.  Engine split per (jt, head) is
table-driven: DVE owns the mask TensorTensor (dominant pass) plus cheap 4x
TensorScalars, Pool takes one head's TensorScalar and some mask slices,
ACT absorbs one head via a relu+exp chain on broadcast s.

Sharding: 8 cores = 4 batches x 2 row-halves (i-slabs of 1024); no
collectives.  Host rotates H rows / A columns so each core's query rows are
local 0..1023 (identical SPMD program) and ships H pre-transposed.
"""

import numpy as np
import ml_dtypes
from contextlib import ExitStack

import concourse.bacc as bacc
import concourse.mybir as mybir
import concourse.tile as tile
from concourse.bass_utils import run_bass_kernel_spmd

B, N, FIN = 4, 2048, 128
KH, DH = 4, 32
P = 128
NI = 1024           # query rows per core
JT = N // P         # 16 j-chunks
WUC = 8 + 512       # warmup tile cols

f32 = mybir.dt.float32
bf16 = mybir.dt.bfloat16
BF = ml_dtypes.bfloat16

_CACHE = {}

# pass1 engine per (jt, k): 'v' = DVE TS, 'p' = Pool TS, 'a' = ACT relu+exp
PASS1_ENG = {}
for _jt in range(JT):
    PASS1_ENG[(_jt, 0)] = "v"
    PASS1_ENG[(_jt, 1)] = "p" if _jt % 4 == 0 else "v"
    PASS1_ENG[(_jt, 2)] = "p"
    PASS1_ENG[(_jt, 3)] = "v" if _jt < 3 else "a"
# jts whose k3 slice of the mask TT runs on Pool instead of DVE
TT_POOL_JT = {1, 4, 7, 10, 13}

JW0 = 5   # initial warmup junk matmuls


def _build_program():
    nc = bacc.Bacc("TRN2", target_bir_lowering=False, debug=False)

    def din(name, shape, dtype):
        return nc.dram_tensor(name, list(shape), dtype, kind="ExternalInput").ap()

    wu_d = din("wu", (P, WUC), bf16)      # tiny first DMA: PE warmup fodder
    CPW = P + 2 * KH + P + N              # [W | WSsrc | WSdst | identb | HT]
    cpack_d = din("cpack", (P, CPW), bf16)
    CP0 = 2 * P + 2 * KH                  # small head of cpack (no HT)
    mT_d = din("maskT", (N, NI), bf16)    # mask (A>0) transposed: [j, i]
    gscrG_d = nc.dram_tensor("gscrG", [KH, NI], bf16).ap()
    gscrS_d = nc.dram_tensor("gscrS", [KH, NI], bf16).ap()
    oaux_d = nc.dram_tensor("oaux", [4, 2, 33, 512], f32,
                            kind="ExternalOutput").ap()

    Exp = mybir.ActivationFunctionType.Exp
    Relu = mybir.ActivationFunctionType.Relu
    Copy = mybir.ActivationFunctionType.Copy
    MULT = mybir.AluOpType.mult
    MAX = mybir.AluOpType.max

    with tile.TileContext(nc) as tc, ExitStack() as ctx:
        const = ctx.enter_context(tc.tile_pool(name="const", bufs=1))
        big = ctx.enter_context(tc.tile_pool(name="big", bufs=1))
        vwork = ctx.enter_context(tc.tile_pool(name="vwork", bufs=5))
        ywork = ctx.enter_context(tc.tile_pool(name="ywork", bufs=4))
        small = ctx.enter_context(tc.tile_pool(name="small", bufs=2))
        ps = ctx.enter_context(tc.tile_pool(name="ps", bufs=1, space="PSUM"))
        pst = ctx.enter_context(tc.tile_pool(name="pst", bufs=1, space="PSUM"))
        pspv = ctx.enter_context(tc.tile_pool(name="pspv", bufs=1, space="PSUM"))

        # ---- inputs; DMA queue order is the critical schedule ----
        wu = const.tile([P, WUC], bf16, tag="wu")
        nc.sync.dma_start(wu[:], wu_d[:])
        cpack = const.tile([P, CPW], bf16, tag="cpack")
        nc.sync.dma_start(cpack[:, 0:CP0], cpack_d[:, 0:CP0])
        nc.sync.dma_start(cpack[:, CP0:], cpack_d[:, CP0:])
        sbW = cpack[:, 0:P]
        wssrc = cpack[:, P:P + KH]
        wsdst = cpack[:, P + KH:P + 2 * KH]
        identb = cpack[:, P + 2 * KH:2 * P + 2 * KH]
        HT = cpack[:, 2 * P + 2 * KH:]  # [fin, n]

        maskT = big.tile([P, JT, NI], bf16, tag="maskT")
        for jt in range(2):  # first two mask chunks up front
            nc.sync.dma_start(maskT[:, jt, :], mT_d[jt * P:(jt + 1) * P, :])

        # PE warm-up junk while cpack lands
        for _ in range(JW0):
            pj = ps.tile([P, 512], f32, tag="stg")
            nc.tensor.matmul(pj[0:8, :], wu[:, 0:8], wu[:, 8:8 + 512],
                             start=True, stop=True)

        # ---- srow/t straight from HT (host folded W into a_src/a_dst) ----
        GrowSb = small.tile([36, NI], bf16, tag="GrowSb", bufs=1)
        for h in range(2):
            psr = ps.tile([P, 512], f32, tag="stg")
            nc.tensor.matmul(psr[0:KH, :], wssrc,
                             HT[:, h * 512:(h + 1) * 512],
                             start=True, stop=True)
            nc.scalar.activation(GrowSb[0:KH, h * 512:(h + 1) * 512],
                                 psr[0:KH, :], Exp, scale=0.8)
            nc.scalar.copy(GrowSb[32:32 + KH, h * 512:(h + 1) * 512],
                           psr[0:KH, :])
        nc.sync.dma_start(gscrG_d[:], GrowSb[0:KH, :])
        Gball = big.tile([P, KH, NI], bf16, tag="Gball")
        Sball = big.tile([P, 1, NI], bf16, tag="Sball")
        for k in [0, 1, 2, 3]:
            nc.sync.dma_start(Gball[:, k, :], gscrG_d[k, :].partition_broadcast(P))
        nc.sync.dma_start(gscrS_d[:], GrowSb[32:32 + KH, :])
        nc.sync.dma_start(Sball[:, 0, :], gscrS_d[3, :].partition_broadcast(P))
        for jt in range(2, 5):
            nc.sync.dma_start(maskT[:, jt, :], mT_d[jt * P:(jt + 1) * P, :])
        # hold the bulk of the mask until the broadcast chain has the DMA
        # engine to itself (readiness-based arbitration would front-run it)
        with tc.tile_wait_until(0.013):
            for jt in range(5, JT):
                nc.sync.dma_start(maskT[:, jt, :], mT_d[jt * P:(jt + 1) * P, :])

        # t-scores for all j: ptt[j, jt*4+k]; HF = exp t, F2 = exp 0.2t
        ptt = pst.tile([P, JT * KH], f32, tag="ptt")
        for jt in range(JT):
            nc.tensor.matmul(ptt[:, jt * KH:(jt + 1) * KH],
                             HT[:, jt * P:(jt + 1) * P], wsdst,
                             start=True, stop=True, skip_group_check=True)
        HFcol = big.tile([P, JT * KH], f32, tag="HFcol")
        F2col = big.tile([P, JT * KH], f32, tag="F2col")
        tcol = big.tile([P, JT * KH], f32, tag="tcol")
        t02col = big.tile([P, JT * KH], f32, tag="t02col")
        nc.scalar.activation(HFcol[:], ptt[:], Exp, scale=1.0)
        nc.scalar.activation(F2col[:], ptt[:], Exp, scale=0.2)
        nc.scalar.copy(tcol[:], ptt[:])
        nc.scalar.activation(t02col[:], ptt[:], Copy, scale=0.2)

        # ---- whf[jt] = [Wh | 1] per head: Wh computed directly in [j, kd]
        # layout (stationary = HT chunk, moving = W); copies PSUM->SBUF are
        # split DVE/ACT; jts >= 4 are emitted inside the main loop.
        pn8 = pst.tile([P, 6, P], f32, tag="pn8")
        whf = []

        def emit_whf(jt):
            nc.tensor.matmul(pn8[:, jt % 6, :], HT[:, jt * P:(jt + 1) * P],
                             sbW, start=True, stop=True)
            wt = big.tile([P, KH, DH + 1], bf16, tag=f"whf{jt}", name=f"whf{jt}")
            nc.gpsimd.memset(wt[:, :, DH:DH + 1], 1.0)
            nc.scalar.copy(
                wt[:, :, 0:DH],
                pn8[:, jt % 6, :].rearrange("p (k d) -> p k d", k=KH),
            )
            whf.append(wt)

        for jt in range(4):
            emit_whf(jt)

        # ---- main loop over j-chunks ----
        pv = [pspv.tile([97, 512], f32, tag=f"pv{q}", name=f"pv{q}")
              for q in range(4)]

        for jt in range(JT):
            if jt + 4 < JT:
                emit_whf(jt + 4)
            v = vwork.tile([P, KH, NI], bf16, tag="v")
            for k in range(KH):
                e = PASS1_ENG[(jt, k)]
                c = jt * KH + k
                if e == "a":
                    rt = small.tile([P, NI], bf16, tag="rt", bufs=3)
                    nc.scalar.activation(rt[:], Sball[:, 0, :], Relu,
                                         bias=tcol[:, c:c + 1], scale=1.0)
                    nc.scalar.activation(v[:, k, :], rt[:], Exp,
                                         bias=t02col[:, c:c + 1], scale=0.8)
                else:
                    eng = nc.vector if e == "v" else nc.gpsimd
                    eng.tensor_scalar(v[:, k, :], Gball[:, k, :],
                                      HFcol[:, c:c + 1], F2col[:, c:c + 1],
                                      MULT, MAX)
            y8 = ywork.tile([P, KH, NI], bf16, tag="y8")

            def pv_mm(k, ib):
                q, r = divmod(k, 2)
                isl = slice(ib * 512, (ib + 1) * 512)
                nc.tensor.matmul(
                    pv[q * 2 + ib][r * 64:r * 64 + 33, :],
                    whf[jt][:, k, :],
                    y8[:, k, isl],
                    start=(jt == 0), stop=(jt == JT - 1),
                    skip_group_check=True,
                )

            if jt < 3:
                # pipeline fill: per-head TT + PV as each broadcast lands
                for k in range(KH):
                    nc.vector.tensor_mul(y8[:, k], v[:, k], maskT[:, jt, :])
                    pv_mm(k, 0)
                    pv_mm(k, 1)
            else:
                if jt in TT_POOL_JT:
                    nc.vector.tensor_mul(
                        y8[:, 0:3], v[:, 0:3],
                        maskT[:, jt, None, :].broadcast_to((P, 3, NI)),
                    )
                    nc.gpsimd.tensor_mul(y8[:, 3], v[:, 3], maskT[:, jt, :])
                else:
                    nc.vector.tensor_mul(
                        y8[:], v[:],
                        maskT[:, jt, None, :].broadcast_to((P, KH, NI)),
                    )
                if jt == JT - 1:
                    for k in (0, 1):
                        pv_mm(k, 0)
                        pv_mm(k, 1)
                    for k in (2, 3):
                        pv_mm(k, 0)
                        pv_mm(k, 1)
                else:
                    for ib in range(2):
                        for k in range(KH):
                            pv_mm(k, ib)

        # ---- epilogue: raw accumulators out; host divides / transposes ----
        for q in range(4):
            ot = small.tile([P, 512], f32, tag="ot", bufs=4)
            eng = nc.vector.tensor_copy if q % 2 == 0 else nc.scalar.copy
            eng(ot[0:33, :], pv[q][0:33, :])
            eng(ot[64:97, :], pv[q][64:97, :])
            nc.sync.dma_start(oaux_d[q, 0], ot[0:33, :])
            nc.sync.dma_start(oaux_d[q, 1], ot[64:97, :])

    nc.compile()
    return nc


def _host_prep(H, A, W, a_src, a_dst):
    """Build the 8 per-core input maps (layout prep + dtype casts only)."""
    Ssrc = np.zeros((FIN, KH), np.float32)
    Sdst = np.zeros((FIN, KH), np.float32)
    for k in range(KH):
        Ssrc[k * DH:(k + 1) * DH, k] = a_src[k]
        Sdst[k * DH:(k + 1) * DH, k] = a_dst[k]
    Wf = W.astype(np.float32)
    WSsrc = Wf @ Ssrc  # [FIN, KH]: s = H @ WSsrc
    WSdst = Wf @ Sdst

    wu_host = np.ones((P, WUC), np.float32)
    wu_host[:, 8:8 + P] = np.eye(P, dtype=np.float32)
    wu_host = wu_host.astype(BF)

    in_maps = []
    for c in range(8):
        b, half = divmod(c, 2)
        i0 = half * NI
        HbT = np.roll(H[b], -i0, axis=0).T  # [FIN, N], j rolled
        maskT = np.ascontiguousarray(
            (np.roll(A[b, i0:i0 + NI, :], -i0, axis=1) > 0).T
        ).astype(BF)
        cpack = np.concatenate(
            [Wf, WSsrc, WSdst, np.eye(P, dtype=np.float32), HbT],
            axis=1,
        ).astype(BF)
        in_maps.append({
            "wu": wu_host,
            "cpack": np.ascontiguousarray(cpack),
            "maskT": maskT,
        })
    return in_maps


def kernel(H, A, W, a_src, a_dst, _want_results=False, _trace=False):
    H = np.asarray(H); A = np.asarray(A); W = np.asarray(W)
    a_src = np.asarray(a_src); a_dst = np.asarray(a_dst)

    if "nc" not in _CACHE:
        _CACHE["nc"] = _build_program()
    nc = _CACHE["nc"]

    in_maps = _host_prep(H, A, W, a_src, a_dst)
    res = run_bass_kernel_spmd(nc, in_maps, list(range(8)), trace=_trace)

    out = np.empty((B, N, KH * DH), np.float32)
    for c in range(8):
        b, half = divmod(c, 2)
        i0 = half * NI
        aux = res.results[c]["oaux"]  # [4, 2, 33, 512] f32
        for q in range(4):
            p, ibb = divmod(q, 2)
            r0 = i0 + ibb * 512
            for h2 in range(2):
                k = 2 * p + h2
                blk = aux[q, h2]  # [33, 512]
                out[b, r0:r0 + 512, k * DH:(k + 1) * DH] = (
                    blk[0:DH] / blk[DH:DH + 1]
                ).T
    if _want_results:
        return out, res
    return out


# revision 19
# speedup vs baseline: 2.0804x; 1.0341x over previous
"""Trainium2 Bass kernel for a dense GAT layer (B=4, N=2048, FIN=128, K=4 heads, D=32).

Math (per batch b):
    Wh = (H @ W).reshape(N, K, D)
    s[i,k] = <Wh[i,k,:], a_src[k,:]>;  t[j,k] = <Wh[j,k,:], a_dst[k,:]>
    e[i,j,k] = leaky_relu(s[i,k] + t[j,k], 0.2), masked to -inf where A[i,j] == 0
    alpha = softmax_j(e);  out[i] = sum_j alpha[i,j,k] * Wh[j,k,:]

Reformulation (exact): with x = s_i + t_j,
    exp(lrelu(x)) = max(exp x, exp 0.2x); the i-side factor exp(0.2 s_i)
    cancels in the softmax.  With G = exp(0.8 s_i), HF_j = exp(t_j),
    F2_j = exp(0.2 t_j), m = (A > 0):
        v[j,i,k] = max(G_ik * HF_jk, F2_jk)          (= F2 * max(GH, 1))
        y[j,i,k] = v * m[j,i]
        num[i,k,:] = sum_j y * Wh[j,k,:];  den[i,k] = sum_j y
        out = num / den
    F2 folds into the tensor-scalar pass (HF as multiplier, F2 as max
    floor), so the PV stationary is raw [Wh | 1] and the denominator falls
    out of the ones column.  s and t come straight from H via host-side
    W@a_src / W@a_dst folds, so the score chain never waits on Wh.

All matmul operands and the score volume are bf16 (PE 1 cyc/row instead of
fp32's 4; DVE 4x mode on the tensor-scalar pass, 2x on the mask multiply).
The mask ships from the host as bf16 {0,1}.  Engine split per (jt, head) is
table-driven: DVE owns the mask TensorTensor (dominant pass) plus cheap 4x
TensorScalars, Pool takes one head's TensorScalar and some mask slices,
ACT absorbs one head via a relu+exp chain on broadcast s.

Sharding: 8 cores = 4 batches x 2 row-halves (i-slabs of 1024); no
collectives.  Host rotates H rows / A columns so each core's query rows are
local 0..1023 (identical SPMD program) and ships H pre-transposed.
"""

import numpy as np
import ml_dtypes
from contextlib import ExitStack

import concourse.bacc as bacc
import concourse.mybir as mybir
import concourse.tile as tile
from concourse.bass_utils import run_bass_kernel_spmd

B, N, FIN = 4, 2048, 128
KH, DH = 4, 32
P = 128
NI = 1024           # query rows per core
JT = N // P         # 16 j-chunks
WUC = 8 + 512       # warmup tile cols

f32 = mybir.dt.float32
bf16 = mybir.dt.bfloat16
BF = ml_dtypes.bfloat16

_CACHE = {}

# pass1 engine per (jt, k): 'v' = DVE TS, 'p' = Pool TS, 'a' = ACT relu+exp
PASS1_ENG = {}
for _jt in range(JT):
    PASS1_ENG[(_jt, 0)] = "v"
    PASS1_ENG[(_jt, 1)] = "p" if _jt % 4 == 0 else "v"
    PASS1_ENG[(_jt, 2)] = "p"
    PASS1_ENG[(_jt, 3)] = "v" if _jt < 3 else "a"
# jts whose k3 slice of the mask TT runs on Pool instead of DVE
TT_POOL_JT = {1, 4, 7, 10, 13}

JW0 = 5   # initial warmup junk matmuls


def _build_program():
    nc = bacc.Bacc("TRN2", target_bir_lowering=False, debug=False)

    def din(name, shape, dtype):
        return nc.dram_tensor(name, list(shape), dtype, kind="ExternalInput").ap()

    wu_d = din("wu", (P, WUC), bf16)      # tiny first DMA: PE warmup fodder
    CPW = P + 2 * KH + P + N              # [W | WSsrc | WSdst | identb | HT]
    cpack_d = din("cpack", (P, CPW), bf16)
    CP0 = 2 * P + 2 * KH                  # small head of cpack (no HT)
    mT_d = din("maskT", (N, NI), bf16)    # mask (A>0) transposed: [j, i]
    gscrG_d = nc.dram_tensor("gscrG", [KH, NI], bf16).ap()
    gscrS_d = nc.dram_tensor("gscrS", [KH, NI], bf16).ap()
    oaux_d = nc.dram_tensor("oaux", [4, 2, 33, 512], f32,
                            kind="ExternalOutput").ap()

    Exp = mybir.ActivationFunctionType.Exp
    Relu = mybir.ActivationFunctionType.Relu
    Copy = mybir.ActivationFunctionType.Copy
    MULT = mybir.AluOpType.mult
    MAX = mybir.AluOpType.max

    with tile.TileContext(nc) as tc, ExitStack() as ctx:
        const = ctx.enter_context(tc.tile_pool(name="const", bufs=1))
        big = ctx.enter_context(tc.tile_pool(name="big", bufs=1))
        vwork = ctx.enter_context(tc.tile_pool(name="vwork", bufs=5))
        ywork = ctx.enter_context(tc.tile_pool(name="ywork", bufs=4))
        small = ctx.enter_context(tc.tile_pool(name="small", bufs=2))
        ps = ctx.enter_context(tc.tile_pool(name="ps", bufs=1, space="PSUM"))
        pnp = ctx.enter_context(tc.tile_pool(name="pnp", bufs=2, space="PSUM"))
        pst = ctx.enter_context(tc.tile_pool(name="pst", bufs=1, space="PSUM"))
        pspv = ctx.enter_context(tc.tile_pool(name="pspv", bufs=1, space="PSUM"))

        # ---- inputs; DMA queue order is the critical schedule ----
        wu = const.tile([P, WUC], bf16, tag="wu")
        nc.sync.dma_start(wu[:], wu_d[:])
        cpack = const.tile([P, CPW], bf16, tag="cpack")
        nc.sync.dma_start(cpack[:, 0:CP0], cpack_d[:, 0:CP0])
        nc.sync.dma_start(cpack[:, CP0:], cpack_d[:, CP0:])
        sbW = cpack[:, 0:P]
        wssrc = cpack[:, P:P + KH]
        wsdst = cpack[:, P + KH:P + 2 * KH]
        identb = cpack[:, P + 2 * KH:2 * P + 2 * KH]
        HT = cpack[:, 2 * P + 2 * KH:]  # [fin, n]

        maskT = big.tile([P, JT, NI], bf16, tag="maskT")
        for jt in range(2):  # first two mask chunks up front
            nc.sync.dma_start(maskT[:, jt, :], mT_d[jt * P:(jt + 1) * P, :])

        # PE warm-up junk while cpack lands
        for _ in range(JW0):
            pj = ps.tile([P, 512], f32, tag="stg")
            nc.tensor.matmul(pj[0:8, :], wu[:, 0:8], wu[:, 8:8 + 512],
                             start=True, stop=True)

        # ---- srow/t straight from HT (host folded W into a_src/a_dst) ----
        GrowSb = small.tile([36, NI], bf16, tag="GrowSb", bufs=1)
        psrs = []
        for h in range(2):
            psr = ps.tile([P, 512], f32, tag="stg")
            nc.tensor.matmul(psr[0:KH, :], wssrc,
                             HT[:, h * 512:(h + 1) * 512],
                             start=True, stop=True)
            nc.scalar.activation(GrowSb[0:KH, h * 512:(h + 1) * 512],
                                 psr[0:KH, :], Exp, scale=0.8)
            psrs.append(psr)
        Gball = big.tile([P, KH, NI], bf16, tag="Gball")
        Sball = big.tile([P, 1, NI], bf16, tag="Sball")
        # head 0 broadcast on (idle) Pool: skips the DRAM round-trip
        nc.gpsimd.partition_broadcast(Gball[:, 0, :], GrowSb[0:1, :])
        nc.sync.dma_start(gscrG_d[:], GrowSb[0:KH, :])
        for k in [1, 2, 3]:
            nc.sync.dma_start(Gball[:, k, :], gscrG_d[k, :].partition_broadcast(P))
        for h in range(2):
            nc.scalar.copy(GrowSb[32:32 + KH, h * 512:(h + 1) * 512],
                           psrs[h][0:KH, :])
        nc.sync.dma_start(gscrS_d[:], GrowSb[32:32 + KH, :])
        nc.sync.dma_start(Sball[:, 0, :], gscrS_d[3, :].partition_broadcast(P))
        for jt in range(2, 5):
            nc.sync.dma_start(maskT[:, jt, :], mT_d[jt * P:(jt + 1) * P, :])
        # hold the bulk of the mask until the broadcast chain has the DMA
        # engine to itself (readiness-based arbitration would front-run it)
        with tc.tile_wait_until(0.013):
            for jt in range(5, JT):
                nc.sync.dma_start(maskT[:, jt, :], mT_d[jt * P:(jt + 1) * P, :])

        # t-scores for all j: ptt[j, jt*4+k]; HF = exp t, F2 = exp 0.2t
        ptt = pst.tile([P, JT * KH], f32, tag="ptt")
        for jt in range(JT):
            nc.tensor.matmul(ptt[:, jt * KH:(jt + 1) * KH],
                             HT[:, jt * P:(jt + 1) * P], wsdst,
                             start=True, stop=True, skip_group_check=True)
        HFcol = big.tile([P, JT * KH], f32, tag="HFcol")
        F2col = big.tile([P, JT * KH], f32, tag="F2col")
        tcol = big.tile([P, JT * KH], f32, tag="tcol")
        t02col = big.tile([P, JT * KH], f32, tag="t02col")
        nc.scalar.activation(HFcol[:], ptt[:], Exp, scale=1.0)
        nc.scalar.activation(F2col[:], ptt[:], Exp, scale=0.2)
        nc.scalar.copy(tcol[:], ptt[:])
        nc.scalar.activation(t02col[:], ptt[:], Copy, scale=0.2)

        # ---- whf[jt] = [Wh | 1] per head: Wh computed directly in [j, kd]
        # layout (stationary = HT chunk, moving = W); copies PSUM->SBUF are
        # split DVE/ACT; jts >= 4 are emitted inside the main loop.
        whf = []

        def emit_whf(jt):
            pn = pnp.tile([P, P], f32, tag="pn")
            nc.tensor.matmul(pn[:], HT[:, jt * P:(jt + 1) * P],
                             sbW, start=True, stop=True)
            wt = big.tile([P, KH, DH + 1], bf16, tag=f"whf{jt}", name=f"whf{jt}")
            nc.gpsimd.memset(wt[:, :, DH:DH + 1], 1.0)
            nc.scalar.copy(
                wt[:, :, 0:DH],
                pn[:].rearrange("p (k d) -> p k d", k=KH),
            )
            whf.append(wt)

        for jt in range(4):
            emit_whf(jt)

        # ---- main loop over j-chunks ----
        pv = [pspv.tile([97, 512], f32, tag=f"pv{q}", name=f"pv{q}")
              for q in range(4)]

        for jt in range(JT):
            if jt + 4 < JT:
                emit_whf(jt + 4)
            v = vwork.tile([P, KH, NI], bf16, tag="v")
            for k in range(KH):
                e = PASS1_ENG[(jt, k)]
                c = jt * KH + k
                if e == "a":
                    rt = small.tile([P, NI], bf16, tag="rt", bufs=3)
                    nc.scalar.activation(rt[:], Sball[:, 0, :], Relu,
                                         bias=tcol[:, c:c + 1], scale=1.0)
                    nc.scalar.activation(v[:, k, :], rt[:], Exp,
                                         bias=t02col[:, c:c + 1], scale=0.8)
                else:
                    eng = nc.vector if e == "v" else nc.gpsimd
                    eng.tensor_scalar(v[:, k, :], Gball[:, k, :],
                                      HFcol[:, c:c + 1], F2col[:, c:c + 1],
                                      MULT, MAX)
            y8 = ywork.tile([P, KH, NI], bf16, tag="y8")

            def pv_mm(k, ib):
                q, r = divmod(k, 2)
                isl = slice(ib * 512, (ib + 1) * 512)
                nc.tensor.matmul(
                    pv[q * 2 + ib][r * 64:r * 64 + 33, :],
                    whf[jt][:, k, :],
                    y8[:, k, isl],
                    start=(jt == 0), stop=(jt == JT - 1),
                    skip_group_check=True,
                )

            if jt < 3:
                # pipeline fill: per-head TT + PV as each broadcast lands
                for k in range(KH):
                    nc.vector.tensor_mul(y8[:, k], v[:, k], maskT[:, jt, :])
                    pv_mm(k, 0)
                    pv_mm(k, 1)
            else:
                if jt in TT_POOL_JT:
                    nc.vector.tensor_mul(
                        y8[:, 0:3], v[:, 0:3],
                        maskT[:, jt, None, :].broadcast_to((P, 3, NI)),
                    )
                    nc.gpsimd.tensor_mul(y8[:, 3], v[:, 3], maskT[:, jt, :])
                else:
                    nc.vector.tensor_mul(
                        y8[:], v[:],
                        maskT[:, jt, None, :].broadcast_to((P, KH, NI)),
                    )
                if jt == JT - 1:
                    for k in (0, 1):
                        pv_mm(k, 0)
                        pv_mm(k, 1)
                    for k in (2, 3):
                        pv_mm(k, 0)
                        pv_mm(k, 1)
                else:
                    for ib in range(2):
                        for k in range(KH):
                            pv_mm(k, ib)

        # ---- epilogue: raw accumulators out; host divides / transposes ----
        for q in range(4):
            ot = small.tile([P, 512], f32, tag="ot", bufs=4)
            eng = nc.vector.tensor_copy if q % 2 == 0 else nc.scalar.copy
            eng(ot[0:33, :], pv[q][0:33, :])
            eng(ot[64:97, :], pv[q][64:97, :])
            nc.sync.dma_start(oaux_d[q, 0], ot[0:33, :])
            nc.sync.dma_start(oaux_d[q, 1], ot[64:97, :])

    nc.compile()
    return nc


def _host_prep(H, A, W, a_src, a_dst):
    """Build the 8 per-core input maps (layout prep + dtype casts only)."""
    Ssrc = np.zeros((FIN, KH), np.float32)
    Sdst = np.zeros((FIN, KH), np.float32)
    for k in range(KH):
        Ssrc[k * DH:(k + 1) * DH, k] = a_src[k]
        Sdst[k * DH:(k + 1) * DH, k] = a_dst[k]
    Wf = W.astype(np.float32)
    WSsrc = Wf @ Ssrc  # [FIN, KH]: s = H @ WSsrc
    WSdst = Wf @ Sdst

    wu_host = np.ones((P, WUC), np.float32)
    wu_host[:, 8:8 + P] = np.eye(P, dtype=np.float32)
    wu_host = wu_host.astype(BF)

    in_maps = []
    for c in range(8):
        b, half = divmod(c, 2)
        i0 = half * NI
        HbT = np.roll(H[b], -i0, axis=0).T  # [FIN, N], j rolled
        maskT = np.ascontiguousarray(
            (np.roll(A[b, i0:i0 + NI, :], -i0, axis=1) > 0).T
        ).astype(BF)
        cpack = np.concatenate(
            [Wf, WSsrc, WSdst, np.eye(P, dtype=np.float32), HbT],
            axis=1,
        ).astype(BF)
        in_maps.append({
            "wu": wu_host,
            "cpack": np.ascontiguousarray(cpack),
            "maskT": maskT,
        })
    return in_maps


def kernel(H, A, W, a_src, a_dst, _want_results=False, _trace=False):
    H = np.asarray(H); A = np.asarray(A); W = np.asarray(W)
    a_src = np.asarray(a_src); a_dst = np.asarray(a_dst)

    if "nc" not in _CACHE:
        _CACHE["nc"] = _build_program()
    nc = _CACHE["nc"]

    in_maps = _host_prep(H, A, W, a_src, a_dst)
    res = run_bass_kernel_spmd(nc, in_maps, list(range(8)), trace=_trace)

    out = np.empty((B, N, KH * DH), np.float32)
    for c in range(8):
        b, half = divmod(c, 2)
        i0 = half * NI
        aux = res.results[c]["oaux"]  # [4, 2, 33, 512] f32
        for q in range(4):
            p, ibb = divmod(q, 2)
            r0 = i0 + ibb * 512
            for h2 in range(2):
                k = 2 * p + h2
                blk = aux[q, h2]  # [33, 512]
                out[b, r0:r0 + 512, k * DH:(k + 1) * DH] = (
                    blk[0:DH] / blk[DH:DH + 1]
                ).T
    if _want_results:
        return out, res
    return out


# revision 20
# speedup vs baseline: 2.1419x; 1.0296x over previous
"""Trainium2 Bass kernel for a dense GAT layer (B=4, N=2048, FIN=128, K=4 heads, D=32).

Math (per batch b):
    Wh = (H @ W).reshape(N, K, D)
    s[i,k] = <Wh[i,k,:], a_src[k,:]>;  t[j,k] = <Wh[j,k,:], a_dst[k,:]>
    e[i,j,k] = leaky_relu(s[i,k] + t[j,k], 0.2), masked to -inf where A[i,j] == 0
    alpha = softmax_j(e);  out[i] = sum_j alpha[i,j,k] * Wh[j,k,:]

Reformulation (exact): with x = s_i + t_j,
    exp(lrelu(x)) = max(exp x, exp 0.2x); the i-side factor exp(0.2 s_i)
    cancels in the softmax.  With G = exp(0.8 s_i), HF_j = exp(t_j),
    F2_j = exp(0.2 t_j), m = (A > 0):
        v[j,i,k] = max(G_ik * HF_jk, F2_jk)          (= F2 * max(GH, 1))
        y[j,i,k] = v * m[j,i]
        num[i,k,:] = sum_j y * Wh[j,k,:];  den[i,k] = sum_j y
        out = num / den
    F2 folds into the tensor-scalar pass (HF as multiplier, F2 as max
    floor), so the PV stationary is raw [Wh | 1] and the denominator falls
    out of the ones column.  s and t come straight from H via host-side
    W@a_src / W@a_dst folds, so the score chain never waits on Wh.

All matmul operands and the score volume are bf16 (PE 1 cyc/row instead of
fp32's 4; DVE 4x mode on the tensor-scalar pass, 2x on the mask multiply).
The mask ships from the host as bf16 {0,1}.  Engine split per (jt, head) is
table-driven: DVE owns the mask TensorTensor (dominant pass) plus cheap 4x
TensorScalars, Pool takes one head's TensorScalar and some mask slices,
ACT absorbs one head via a relu+exp chain on broadcast s.

Sharding: 8 cores = 4 batches x 2 row-halves (i-slabs of 1024); no
collectives.  Host rotates H rows / A columns so each core's query rows are
local 0..1023 (identical SPMD program) and ships H pre-transposed.
"""

import numpy as np
import ml_dtypes
from contextlib import ExitStack

import concourse.bacc as bacc
import concourse.mybir as mybir
import concourse.tile as tile
from concourse.bass_utils import run_bass_kernel_spmd

B, N, FIN = 4, 2048, 128
KH, DH = 4, 32
P = 128
NI = 1024           # query rows per core
JT = N // P         # 16 j-chunks
WUC = 8 + 512       # warmup tile cols

f32 = mybir.dt.float32
bf16 = mybir.dt.bfloat16
BF = ml_dtypes.bfloat16

_CACHE = {}

# pass1 engine per (jt, k): 'v' = DVE TS, 'p' = Pool TS, 'a' = ACT relu+exp
PASS1_ENG = {}
for _jt in range(JT):
    PASS1_ENG[(_jt, 0)] = "v"
    PASS1_ENG[(_jt, 1)] = "p" if _jt % 4 == 0 else "v"
    PASS1_ENG[(_jt, 2)] = "p"
    PASS1_ENG[(_jt, 3)] = "v" if _jt < 3 else "a"
for _jt in (11, 13, 14, 15):
    PASS1_ENG[(_jt, 1)] = "a"
# jts whose k3 slice of the mask TT runs on Pool instead of DVE
TT_POOL_JT = {1, 4, 7, 10, 12, 13, 14, 15}

JW0 = 5   # initial warmup junk matmuls


def _build_program():
    nc = bacc.Bacc("TRN2", target_bir_lowering=False, debug=False)

    def din(name, shape, dtype):
        return nc.dram_tensor(name, list(shape), dtype, kind="ExternalInput").ap()

    wu_d = din("wu", (P, WUC), bf16)      # tiny first DMA: PE warmup fodder
    CPW = P + 2 * KH + P + N              # [W | WSsrc | WSdst | identb | HT]
    cpack_d = din("cpack", (P, CPW), bf16)
    CP0 = 2 * P + 2 * KH                  # small head of cpack (no HT)
    mT_d = din("maskT", (N, NI), bf16)    # mask (A>0) transposed: [j, i]
    gscrG_d = nc.dram_tensor("gscrG", [KH, NI], bf16).ap()
    gscrS_d = nc.dram_tensor("gscrS", [KH, NI], bf16).ap()
    oaux_d = nc.dram_tensor("oaux", [2, 33, 4, 512], f32,
                            kind="ExternalOutput").ap()

    Exp = mybir.ActivationFunctionType.Exp
    Relu = mybir.ActivationFunctionType.Relu
    Copy = mybir.ActivationFunctionType.Copy
    MULT = mybir.AluOpType.mult
    MAX = mybir.AluOpType.max

    with tile.TileContext(nc) as tc, ExitStack() as ctx:
        const = ctx.enter_context(tc.tile_pool(name="const", bufs=1))
        big = ctx.enter_context(tc.tile_pool(name="big", bufs=1))
        vwork = ctx.enter_context(tc.tile_pool(name="vwork", bufs=5))
        ywork = ctx.enter_context(tc.tile_pool(name="ywork", bufs=4))
        small = ctx.enter_context(tc.tile_pool(name="small", bufs=2))
        ps = ctx.enter_context(tc.tile_pool(name="ps", bufs=1, space="PSUM"))
        pnp = ctx.enter_context(tc.tile_pool(name="pnp", bufs=2, space="PSUM"))
        pst = ctx.enter_context(tc.tile_pool(name="pst", bufs=1, space="PSUM"))
        pspv = ctx.enter_context(tc.tile_pool(name="pspv", bufs=1, space="PSUM"))

        # ---- inputs; DMA queue order is the critical schedule ----
        wu = const.tile([P, WUC], bf16, tag="wu")
        nc.sync.dma_start(wu[:], wu_d[:])
        cpack = const.tile([P, CPW], bf16, tag="cpack")
        nc.sync.dma_start(cpack[:, 0:CP0], cpack_d[:, 0:CP0])
        nc.sync.dma_start(cpack[:, CP0:], cpack_d[:, CP0:])
        sbW = cpack[:, 0:P]
        wssrc = cpack[:, P:P + KH]
        wsdst = cpack[:, P + KH:P + 2 * KH]
        identb = cpack[:, P + 2 * KH:2 * P + 2 * KH]
        HT = cpack[:, 2 * P + 2 * KH:]  # [fin, n]

        maskT = big.tile([P, JT, NI], bf16, tag="maskT")
        for jt in range(2):  # first two mask chunks up front
            nc.sync.dma_start(maskT[:, jt, :], mT_d[jt * P:(jt + 1) * P, :])

        # PE warm-up junk while cpack lands
        for _ in range(JW0):
            pj = ps.tile([P, 512], f32, tag="stg")
            nc.tensor.matmul(pj[0:8, :], wu[:, 0:8], wu[:, 8:8 + 512],
                             start=True, stop=True)

        # ---- srow/t straight from HT (host folded W into a_src/a_dst) ----
        GrowSb = small.tile([36, NI], bf16, tag="GrowSb", bufs=1)
        psrs = []
        for h in range(2):
            psr = ps.tile([P, 512], f32, tag="stg")
            nc.tensor.matmul(psr[0:KH, :], wssrc,
                             HT[:, h * 512:(h + 1) * 512],
                             start=True, stop=True)
            nc.scalar.activation(GrowSb[0:KH, h * 512:(h + 1) * 512],
                                 psr[0:KH, :], Exp, scale=0.8)
            psrs.append(psr)
        Gball = big.tile([P, KH, NI], bf16, tag="Gball")
        Sball = big.tile([P, 2, NI], bf16, tag="Sball")
        # head 0 broadcast on (idle) Pool: skips the DRAM round-trip
        nc.gpsimd.partition_broadcast(Gball[:, 0, :], GrowSb[0:1, :])
        nc.sync.dma_start(gscrG_d[:], GrowSb[0:KH, :])
        for k in [1, 2, 3]:
            nc.sync.dma_start(Gball[:, k, :], gscrG_d[k, :].partition_broadcast(P))
        for h in range(2):
            nc.scalar.copy(GrowSb[32:32 + KH, h * 512:(h + 1) * 512],
                           psrs[h][0:KH, :])
        nc.sync.dma_start(gscrS_d[:], GrowSb[32:32 + KH, :])
        nc.sync.dma_start(Sball[:, 1, :], gscrS_d[3, :].partition_broadcast(P))
        nc.sync.dma_start(Sball[:, 0, :], gscrS_d[1, :].partition_broadcast(P))
        for jt in range(2, 5):
            nc.sync.dma_start(maskT[:, jt, :], mT_d[jt * P:(jt + 1) * P, :])
        # hold the bulk of the mask until the broadcast chain has the DMA
        # engine to itself (readiness-based arbitration would front-run it)
        with tc.tile_wait_until(0.013):
            for jt in range(5, JT):
                nc.sync.dma_start(maskT[:, jt, :], mT_d[jt * P:(jt + 1) * P, :])

        # t-scores for all j: ptt[j, jt*4+k]; HF = exp t, F2 = exp 0.2t
        ptt = pst.tile([P, JT * KH], f32, tag="ptt")
        for jt in range(JT):
            nc.tensor.matmul(ptt[:, jt * KH:(jt + 1) * KH],
                             HT[:, jt * P:(jt + 1) * P], wsdst,
                             start=True, stop=True, skip_group_check=True)
        HFcol = big.tile([P, JT * KH], f32, tag="HFcol")
        F2col = big.tile([P, JT * KH], f32, tag="F2col")
        tcol = big.tile([P, JT * KH], f32, tag="tcol")
        t02col = big.tile([P, JT * KH], f32, tag="t02col")
        nc.scalar.activation(HFcol[:], ptt[:], Exp, scale=1.0)
        nc.scalar.activation(F2col[:], ptt[:], Exp, scale=0.2)
        nc.scalar.copy(tcol[:], ptt[:])
        nc.scalar.activation(t02col[:], ptt[:], Copy, scale=0.2)

        # ---- whf[jt] = [Wh | 1] per head: Wh computed directly in [j, kd]
        # layout (stationary = HT chunk, moving = W); copies PSUM->SBUF are
        # split DVE/ACT; jts >= 4 are emitted inside the main loop.
        whf = []

        def emit_whf(jt):
            pn = pnp.tile([P, P], f32, tag="pn")
            nc.tensor.matmul(pn[:], HT[:, jt * P:(jt + 1) * P],
                             sbW, start=True, stop=True)
            wt = big.tile([P, KH, DH + 1], bf16, tag=f"whf{jt}", name=f"whf{jt}")
            nc.gpsimd.memset(wt[:, :, DH:DH + 1], 1.0)
            nc.scalar.copy(
                wt[:, :, 0:DH],
                pn[:].rearrange("p (k d) -> p k d", k=KH),
            )
            whf.append(wt)

        for jt in range(4):
            emit_whf(jt)

        # ---- main loop over j-chunks ----
        pv = [pspv.tile([97, 512], f32, tag=f"pv{q}", name=f"pv{q}")
              for q in range(4)]

        for jt in range(JT):
            if jt + 4 < JT:
                emit_whf(jt + 4)
            v = vwork.tile([P, KH, NI], bf16, tag="v")
            for k in range(KH):
                e = PASS1_ENG[(jt, k)]
                c = jt * KH + k
                if e == "a":
                    rt = small.tile([P, NI], bf16, tag="rt", bufs=3)
                    srow_idx = 0 if k == 1 else 1
                    nc.scalar.activation(rt[:], Sball[:, srow_idx, :], Relu,
                                         bias=tcol[:, c:c + 1], scale=1.0)
                    nc.scalar.activation(v[:, k, :], rt[:], Exp,
                                         bias=t02col[:, c:c + 1], scale=0.8)
                else:
                    eng = nc.vector if e == "v" else nc.gpsimd
                    eng.tensor_scalar(v[:, k, :], Gball[:, k, :],
                                      HFcol[:, c:c + 1], F2col[:, c:c + 1],
                                      MULT, MAX)
            y8 = ywork.tile([P, KH, NI], bf16, tag="y8")

            def pv_mm(k, ib):
                q, r = divmod(k, 2)
                isl = slice(ib * 512, (ib + 1) * 512)
                nc.tensor.matmul(
                    pv[q * 2 + ib][r * 64:r * 64 + 33, :],
                    whf[jt][:, k, :],
                    y8[:, k, isl],
                    start=(jt == 0), stop=(jt == JT - 1),
                    skip_group_check=True,
                )

            if jt < 3:
                # pipeline fill: per-head TT + PV as each broadcast lands
                for k in range(KH):
                    nc.vector.tensor_mul(y8[:, k], v[:, k], maskT[:, jt, :])
                    pv_mm(k, 0)
                    pv_mm(k, 1)
            else:
                if jt in TT_POOL_JT:
                    nc.vector.tensor_mul(
                        y8[:, 0:3], v[:, 0:3],
                        maskT[:, jt, None, :].broadcast_to((P, 3, NI)),
                    )
                    nc.gpsimd.tensor_mul(y8[:, 3], v[:, 3], maskT[:, jt, :])
                else:
                    nc.vector.tensor_mul(
                        y8[:], v[:],
                        maskT[:, jt, None, :].broadcast_to((P, KH, NI)),
                    )
                if jt == JT - 1:
                    for k in (0, 1):
                        pv_mm(k, 0)
                        pv_mm(k, 1)
                    for k in (2, 3):
                        pv_mm(k, 0)
                        pv_mm(k, 1)
                else:
                    for ib in range(2):
                        for k in range(KH):
                            pv_mm(k, ib)

        # ---- epilogue: raw accumulators out; host divides / transposes ----
        otall = small.tile([P, 4, 512], f32, tag="otall", bufs=1)
        for q in range(4):
            eng = nc.vector.tensor_copy if q % 2 == 0 else nc.scalar.copy
            eng(otall[0:33, q, :], pv[q][0:33, :])
            eng(otall[64:97, q, :], pv[q][64:97, :])
        nc.sync.dma_start(oaux_d[0], otall[0:33, :, :])
        nc.sync.dma_start(oaux_d[1], otall[64:97, :, :])

    nc.compile()
    return nc


def _host_prep(H, A, W, a_src, a_dst):
    """Build the 8 per-core input maps (layout prep + dtype casts only)."""
    Ssrc = np.zeros((FIN, KH), np.float32)
    Sdst = np.zeros((FIN, KH), np.float32)
    for k in range(KH):
        Ssrc[k * DH:(k + 1) * DH, k] = a_src[k]
        Sdst[k * DH:(k + 1) * DH, k] = a_dst[k]
    Wf = W.astype(np.float32)
    WSsrc = Wf @ Ssrc  # [FIN, KH]: s = H @ WSsrc
    WSdst = Wf @ Sdst

    wu_host = np.ones((P, WUC), np.float32)
    wu_host[:, 8:8 + P] = np.eye(P, dtype=np.float32)
    wu_host = wu_host.astype(BF)

    in_maps = []
    for c in range(8):
        b, half = divmod(c, 2)
        i0 = half * NI
        HbT = np.roll(H[b], -i0, axis=0).T  # [FIN, N], j rolled
        maskT = np.ascontiguousarray(
            (np.roll(A[b, i0:i0 + NI, :], -i0, axis=1) > 0).T
        ).astype(BF)
        cpack = np.concatenate(
            [Wf, WSsrc, WSdst, np.eye(P, dtype=np.float32), HbT],
            axis=1,
        ).astype(BF)
        in_maps.append({
            "wu": wu_host,
            "cpack": np.ascontiguousarray(cpack),
            "maskT": maskT,
        })
    return in_maps


def kernel(H, A, W, a_src, a_dst, _want_results=False, _trace=False):
    H = np.asarray(H); A = np.asarray(A); W = np.asarray(W)
    a_src = np.asarray(a_src); a_dst = np.asarray(a_dst)

    if "nc" not in _CACHE:
        _CACHE["nc"] = _build_program()
    nc = _CACHE["nc"]

    in_maps = _host_prep(H, A, W, a_src, a_dst)
    res = run_bass_kernel_spmd(nc, in_maps, list(range(8)), trace=_trace)

    out = np.empty((B, N, KH * DH), np.float32)
    for c in range(8):
        b, half = divmod(c, 2)
        i0 = half * NI
        aux = res.results[c]["oaux"]  # [2, 33, 4, 512] f32
        for q in range(4):
            p, ibb = divmod(q, 2)
            r0 = i0 + ibb * 512
            for h2 in range(2):
                k = 2 * p + h2
                blk = aux[h2, :, q, :]  # [33, 512]
                out[b, r0:r0 + 512, k * DH:(k + 1) * DH] = (
                    blk[0:DH] / blk[DH:DH + 1]
                ).T
    if _want_results:
        return out, res
    return out


# revision 21
# speedup vs baseline: 2.2355x; 1.0437x over previous
"""Trainium2 Bass kernel for a dense GAT layer (B=4, N=2048, FIN=128, K=4 heads, D=32).

Math (per batch b):
    Wh = (H @ W).reshape(N, K, D)
    s[i,k] = <Wh[i,k,:], a_src[k,:]>;  t[j,k] = <Wh[j,k,:], a_dst[k,:]>
    e[i,j,k] = leaky_relu(s[i,k] + t[j,k], 0.2), masked to -inf where A[i,j] == 0
    alpha = softmax_j(e);  out[i] = sum_j alpha[i,j,k] * Wh[j,k,:]

Reformulation (exact): with x = s_i + t_j,
    exp(lrelu(x)) = max(exp x, exp 0.2x); the i-side factor exp(0.2 s_i)
    cancels in the softmax.  With G = exp(0.8 s_i), HF_j = exp(t_j),
    F2_j = exp(0.2 t_j), m = (A > 0):
        v[j,i,k] = max(G_ik * HF_jk, F2_jk)          (= F2 * max(GH, 1))
        y[j,i,k] = v * m[j,i]
        num[i,k,:] = sum_j y * Wh[j,k,:];  den[i,k] = sum_j y
        out = num / den
    F2 folds into the tensor-scalar pass (HF as multiplier, F2 as max
    floor), so the PV stationary is raw [Wh | 1] and the denominator falls
    out of the ones column.  s and t come straight from H via host-side
    W@a_src / W@a_dst folds, so the score chain never waits on Wh.

All matmul operands and the score volume are bf16 (PE 1 cyc/row instead of
fp32's 4; DVE 4x mode on the tensor-scalar pass, 2x on the mask multiply).
The mask ships from the host as bf16 {0,1}.  Engine split per (jt, head) is
table-driven: DVE owns the mask TensorTensor (dominant pass) plus cheap 4x
TensorScalars, Pool takes one head's TensorScalar and some mask slices,
ACT absorbs one head via a relu+exp chain on broadcast s.

Sharding: 8 cores = 4 batches x 2 row-halves (i-slabs of 1024); no
collectives.  Host rotates H rows / A columns so each core's query rows are
local 0..1023 (identical SPMD program) and ships H pre-transposed.
"""

import numpy as np
import ml_dtypes
from contextlib import ExitStack

import concourse.bacc as bacc
import concourse.mybir as mybir
import concourse.tile as tile
from concourse.bass_utils import run_bass_kernel_spmd

B, N, FIN = 4, 2048, 128
KH, DH = 4, 32
P = 128
NI = 1024           # query rows per core
JT = N // P         # 16 j-chunks
WUC = 8 + 512       # warmup tile cols

f32 = mybir.dt.float32
bf16 = mybir.dt.bfloat16
BF = ml_dtypes.bfloat16

_CACHE = {}

# pass1 engine per (jt, k): 'v' = DVE TS, 'p' = Pool TS, 'a' = ACT relu+exp
PASS1_ENG = {}
for _jt in range(JT):
    PASS1_ENG[(_jt, 0)] = "v"
    PASS1_ENG[(_jt, 1)] = "p" if _jt % 4 == 0 else "v"
    PASS1_ENG[(_jt, 2)] = "p"
    PASS1_ENG[(_jt, 3)] = "v" if _jt < 3 else "a"
for _jt in (13, 15):
    PASS1_ENG[(_jt, 1)] = "a"
for _jt in (6, 10):
    PASS1_ENG[(_jt, 2)] = "a"
# jts whose k3 slice of the mask TT runs on Pool instead of DVE
TT_POOL_JT = {1, 4, 7, 10, 12, 14}

JW0 = 5   # initial warmup junk matmuls


def _build_program():
    nc = bacc.Bacc("TRN2", target_bir_lowering=False, debug=False)

    def din(name, shape, dtype):
        return nc.dram_tensor(name, list(shape), dtype, kind="ExternalInput").ap()

    wu_d = din("wu", (P, WUC), bf16)      # tiny first DMA: PE warmup fodder
    CPW = P + 2 * KH + P + N              # [W | WSsrc | WSdst | identb | HT]
    cpack_d = din("cpack", (P, CPW), bf16)
    CP0 = 2 * P + 2 * KH                  # small head of cpack (no HT)
    mT_d = din("maskT", (N, NI), bf16)    # mask (A>0) transposed: [j, i]
    gscrG_d = nc.dram_tensor("gscrG", [KH, NI], bf16).ap()
    gscrS_d = nc.dram_tensor("gscrS", [KH, NI], bf16).ap()
    oaux_d = nc.dram_tensor("oaux", [2, 33, 4, 512], f32,
                            kind="ExternalOutput").ap()

    Exp = mybir.ActivationFunctionType.Exp
    Relu = mybir.ActivationFunctionType.Relu
    Copy = mybir.ActivationFunctionType.Copy
    MULT = mybir.AluOpType.mult
    MAX = mybir.AluOpType.max

    with tile.TileContext(nc) as tc, ExitStack() as ctx:
        const = ctx.enter_context(tc.tile_pool(name="const", bufs=1))
        big = ctx.enter_context(tc.tile_pool(name="big", bufs=1))
        vwork = ctx.enter_context(tc.tile_pool(name="vwork", bufs=5))
        ywork = ctx.enter_context(tc.tile_pool(name="ywork", bufs=4))
        small = ctx.enter_context(tc.tile_pool(name="small", bufs=2))
        ps = ctx.enter_context(tc.tile_pool(name="ps", bufs=1, space="PSUM"))
        pnp = ctx.enter_context(tc.tile_pool(name="pnp", bufs=2, space="PSUM"))
        pst = ctx.enter_context(tc.tile_pool(name="pst", bufs=1, space="PSUM"))
        pspv = ctx.enter_context(tc.tile_pool(name="pspv", bufs=1, space="PSUM"))

        # ---- inputs; DMA queue order is the critical schedule ----
        wu = const.tile([P, WUC], bf16, tag="wu")
        nc.sync.dma_start(wu[:], wu_d[:])
        cpack = const.tile([P, CPW], bf16, tag="cpack")
        nc.sync.dma_start(cpack[:, 0:CP0], cpack_d[:, 0:CP0])
        nc.sync.dma_start(cpack[:, CP0:], cpack_d[:, CP0:])
        sbW = cpack[:, 0:P]
        wssrc = cpack[:, P:P + KH]
        wsdst = cpack[:, P + KH:P + 2 * KH]
        identb = cpack[:, P + 2 * KH:2 * P + 2 * KH]
        HT = cpack[:, 2 * P + 2 * KH:]  # [fin, n]

        maskT = big.tile([P, JT, NI], bf16, tag="maskT")
        for jt in range(2):  # first two mask chunks up front
            nc.sync.dma_start(maskT[:, jt, :], mT_d[jt * P:(jt + 1) * P, :])

        # PE warm-up junk while cpack lands
        for _ in range(JW0):
            pj = ps.tile([P, 512], f32, tag="stg")
            nc.tensor.matmul(pj[0:8, :], wu[:, 0:8], wu[:, 8:8 + 512],
                             start=True, stop=True)

        # ---- srow/t straight from HT (host folded W into a_src/a_dst) ----
        GrowSb = small.tile([36, NI], bf16, tag="GrowSb", bufs=1)
        psrs = []
        for h in range(2):
            psr = ps.tile([P, 512], f32, tag="stg")
            nc.tensor.matmul(psr[0:KH, :], wssrc,
                             HT[:, h * 512:(h + 1) * 512],
                             start=True, stop=True)
            nc.scalar.activation(GrowSb[0:KH, h * 512:(h + 1) * 512],
                                 psr[0:KH, :], Exp, scale=0.8)
            psrs.append(psr)
        Gball = big.tile([P, KH, NI], bf16, tag="Gball")
        Sball = big.tile([P, 3, NI], bf16, tag="Sball")
        # head 0 broadcast on (idle) Pool: skips the DRAM round-trip
        nc.gpsimd.partition_broadcast(Gball[:, 0, :], GrowSb[0:1, :])
        nc.sync.dma_start(gscrG_d[:], GrowSb[0:KH, :])
        for k in [1, 2, 3]:
            nc.sync.dma_start(Gball[:, k, :], gscrG_d[k, :].partition_broadcast(P))
        for h in range(2):
            nc.scalar.copy(GrowSb[32:32 + KH, h * 512:(h + 1) * 512],
                           psrs[h][0:KH, :])
        nc.sync.dma_start(gscrS_d[:], GrowSb[32:32 + KH, :])
        nc.sync.dma_start(Sball[:, 2, :], gscrS_d[3, :].partition_broadcast(P))
        nc.sync.dma_start(Sball[:, 0, :], gscrS_d[1, :].partition_broadcast(P))
        nc.sync.dma_start(Sball[:, 1, :], gscrS_d[2, :].partition_broadcast(P))
        for jt in range(2, 5):
            nc.sync.dma_start(maskT[:, jt, :], mT_d[jt * P:(jt + 1) * P, :])
        # hold the bulk of the mask until the broadcast chain has the DMA
        # engine to itself (readiness-based arbitration would front-run it)
        with tc.tile_wait_until(0.013):
            for jt in range(5, JT):
                nc.sync.dma_start(maskT[:, jt, :], mT_d[jt * P:(jt + 1) * P, :])

        # t-scores for all j: ptt[j, jt*4+k]; HF = exp t, F2 = exp 0.2t
        ptt = pst.tile([P, JT * KH], f32, tag="ptt")
        for jt in range(JT):
            nc.tensor.matmul(ptt[:, jt * KH:(jt + 1) * KH],
                             HT[:, jt * P:(jt + 1) * P], wsdst,
                             start=True, stop=True, skip_group_check=True)
        HFcol = big.tile([P, JT * KH], f32, tag="HFcol")
        F2col = big.tile([P, JT * KH], f32, tag="F2col")
        tcol = big.tile([P, JT * KH], f32, tag="tcol")
        t02col = big.tile([P, JT * KH], f32, tag="t02col")
        nc.scalar.activation(HFcol[:], ptt[:], Exp, scale=1.0)
        nc.scalar.activation(F2col[:], ptt[:], Exp, scale=0.2)
        nc.scalar.copy(tcol[:], ptt[:])
        nc.scalar.activation(t02col[:], ptt[:], Copy, scale=0.2)

        # ---- whf[jt] = [Wh | 1] per head: Wh computed directly in [j, kd]
        # layout (stationary = HT chunk, moving = W); copies PSUM->SBUF are
        # split DVE/ACT; jts >= 4 are emitted inside the main loop.
        whf = []

        def emit_whf(jt):
            pn = pnp.tile([P, P], f32, tag="pn")
            nc.tensor.matmul(pn[:], HT[:, jt * P:(jt + 1) * P],
                             sbW, start=True, stop=True)
            wt = big.tile([P, KH, DH + 1], bf16, tag=f"whf{jt}", name=f"whf{jt}")
            nc.gpsimd.memset(wt[:, :, DH:DH + 1], 1.0)
            nc.scalar.copy(
                wt[:, :, 0:DH],
                pn[:].rearrange("p (k d) -> p k d", k=KH),
            )
            whf.append(wt)

        for jt in range(4):
            emit_whf(jt)

        # ---- main loop over j-chunks ----
        pv = [pspv.tile([97, 512], f32, tag=f"pv{q}", name=f"pv{q}")
              for q in range(4)]

        for jt in range(JT):
            if jt + 4 < JT:
                emit_whf(jt + 4)
            v = vwork.tile([P, KH, NI], bf16, tag="v")
            for k in range(KH):
                e = PASS1_ENG[(jt, k)]
                c = jt * KH + k
                if e == "a":
                    rt = small.tile([P, NI], bf16, tag="rt", bufs=3)
                    srow_idx = k - 1
                    nc.scalar.activation(rt[:], Sball[:, srow_idx, :], Relu,
                                         bias=tcol[:, c:c + 1], scale=1.0)
                    nc.scalar.activation(v[:, k, :], rt[:], Exp,
                                         bias=t02col[:, c:c + 1], scale=0.8)
                else:
                    eng = nc.vector if e == "v" else nc.gpsimd
                    eng.tensor_scalar(v[:, k, :], Gball[:, k, :],
                                      HFcol[:, c:c + 1], F2col[:, c:c + 1],
                                      MULT, MAX)
            y8 = ywork.tile([P, KH, NI], bf16, tag="y8")

            def pv_mm(k, ib):
                q, r = divmod(k, 2)
                isl = slice(ib * 512, (ib + 1) * 512)
                nc.tensor.matmul(
                    pv[q * 2 + ib][r * 64:r * 64 + 33, :],
                    whf[jt][:, k, :],
                    y8[:, k, isl],
                    start=(jt == 0), stop=(jt == JT - 1),
                    skip_group_check=True,
                )

            if jt < 3:
                # pipeline fill: per-head TT + PV as each broadcast lands
                for k in range(KH):
                    nc.vector.tensor_mul(y8[:, k], v[:, k], maskT[:, jt, :])
                    pv_mm(k, 0)
                    pv_mm(k, 1)
            else:
                if jt in TT_POOL_JT:
                    nc.vector.tensor_mul(
                        y8[:, 0:3], v[:, 0:3],
                        maskT[:, jt, None, :].broadcast_to((P, 3, NI)),
                    )
                    nc.gpsimd.tensor_mul(y8[:, 3], v[:, 3], maskT[:, jt, :])
                else:
                    nc.vector.tensor_mul(
                        y8[:], v[:],
                        maskT[:, jt, None, :].broadcast_to((P, KH, NI)),
                    )
                if jt == JT - 1:
                    for k in (0, 1):
                        pv_mm(k, 0)
                        pv_mm(k, 1)
                    for k in (2, 3):
                        pv_mm(k, 0)
                        pv_mm(k, 1)
                else:
                    for ib in range(2):
                        for k in range(KH):
                            pv_mm(k, ib)

        # ---- epilogue: raw accumulators out; host divides / transposes ----
        otall = small.tile([P, 4, 512], f32, tag="otall", bufs=1)
        for q in range(4):
            eng = nc.vector.tensor_copy if q % 2 == 0 else nc.scalar.copy
            eng(otall[0:33, q, :], pv[q][0:33, :])
        nc.sync.dma_start(oaux_d[0], otall[0:33, :, :])
        for q in range(4):
            eng = nc.vector.tensor_copy if q % 2 == 0 else nc.scalar.copy
            eng(otall[64:97, q, :], pv[q][64:97, :])
        nc.sync.dma_start(oaux_d[1], otall[64:97, :, :])

    nc.compile()
    return nc


def _host_prep(H, A, W, a_src, a_dst):
    """Build the 8 per-core input maps (layout prep + dtype casts only)."""
    Ssrc = np.zeros((FIN, KH), np.float32)
    Sdst = np.zeros((FIN, KH), np.float32)
    for k in range(KH):
        Ssrc[k * DH:(k + 1) * DH, k] = a_src[k]
        Sdst[k * DH:(k + 1) * DH, k] = a_dst[k]
    Wf = W.astype(np.float32)
    WSsrc = Wf @ Ssrc  # [FIN, KH]: s = H @ WSsrc
    WSdst = Wf @ Sdst

    wu_host = np.ones((P, WUC), np.float32)
    wu_host[:, 8:8 + P] = np.eye(P, dtype=np.float32)
    wu_host = wu_host.astype(BF)

    in_maps = []
    for c in range(8):
        b, half = divmod(c, 2)
        i0 = half * NI
        HbT = np.roll(H[b], -i0, axis=0).T  # [FIN, N], j rolled
        maskT = np.ascontiguousarray(
            (np.roll(A[b, i0:i0 + NI, :], -i0, axis=1) > 0).T
        ).astype(BF)
        cpack = np.concatenate(
            [Wf, WSsrc, WSdst, np.eye(P, dtype=np.float32), HbT],
            axis=1,
        ).astype(BF)
        in_maps.append({
            "wu": wu_host,
            "cpack": np.ascontiguousarray(cpack),
            "maskT": maskT,
        })
    return in_maps


def kernel(H, A, W, a_src, a_dst, _want_results=False, _trace=False):
    H = np.asarray(H); A = np.asarray(A); W = np.asarray(W)
    a_src = np.asarray(a_src); a_dst = np.asarray(a_dst)

    if "nc" not in _CACHE:
        _CACHE["nc"] = _build_program()
    nc = _CACHE["nc"]

    in_maps = _host_prep(H, A, W, a_src, a_dst)
    res = run_bass_kernel_spmd(nc, in_maps, list(range(8)), trace=_trace)

    out = np.empty((B, N, KH * DH), np.float32)
    for c in range(8):
        b, half = divmod(c, 2)
        i0 = half * NI
        aux = res.results[c]["oaux"]  # [2, 33, 4, 512] f32
        for q in range(4):
            p, ibb = divmod(q, 2)
            r0 = i0 + ibb * 512
            for h2 in range(2):
                k = 2 * p + h2
                blk = aux[h2, :, q, :]  # [33, 512]
                out[b, r0:r0 + 512, k * DH:(k + 1) * DH] = (
                    blk[0:DH] / blk[DH:DH + 1]
                ).T
    if _want_results:
        return out, res
    return out
